# revision 1
# baseline (speedup 1.0000x reference)
"""Distributed 2-layer GAT (nn_AlignHead) on 8 TRN2 NeuronCores.

Strategy: shard nodes (dst) contiguously across 8 cores. Per core:
  Phase A: LayerNorm + h@W1_ext dense matmul -> per-node table rows
           [g1 (c-major, 512) | a_src1 (8) | pad] bf16; AllGather tables.
  Phase B: per dst-tile (128 dsts): dma_gather edge src rows (4 SWDGE
           queues), segment-softmax via indicator-matmul machinery:
           P[e,s] built on DVE (is_equal vs pre-expanded iota), Q = P^T via
           TensorE transpose, a_dst broadcast via Q-matmul, p =
           max(exp(z), exp(0.2 z)) (= exp(leaky_relu(z))), weighted
           aggregation + denominator via per-window matmuls into PSUM,
           normalize, ELU -> h2, dense h2@W2_ext -> table2; AllGather.
  Phase C: conv2 edge phase (1 head replicated to 8 pseudo-heads of 32),
           normalize -> final output rows.

Self-contained: hardcodes the problem shapes; compiles on first call.
"""
import sys
import types

import numpy as np
import ml_dtypes

# ---------------------------------------------------------------- constants
NCORE = 8
N = 50000
E = 500000
D = 256
H1, C1 = 8, 64
DH = 512            # H1*C1
NEG = 0.2
EPS = 1e-5
NLOC = 6250         # nodes per core
NPAD = 6272         # 49*128
T = 49              # dst tiles per core
ROWS = NPAD * NCORE  # 50176 global (padded) table rows
LO = 32768          # int16 gather row limit
ELEM1 = 640         # bf16 elems per conv1 table row (1280 B)
ELEM2 = 384         # bf16 elems per conv2 table row (768 B)
NQ = 4              # swdge queues
# AG chunks double as the int16 gather-range split: chunk q's global table
# [8*(CB[q+1]-CB[q]) rows] fits int16 indexing entirely.
CHUNKB = [0, 3200, 6272]   # AG chunk boundaries (local rows)
CHUNKT = [0, 25, 49]       # tile boundaries per chunk
BF = ml_dtypes.bfloat16

_cache = {}


def _install_ntff_hook():
    if "antenv.axon_hooks" in sys.modules:
        return
    try:
        import antenv
        mod = types.ModuleType("antenv.axon_hooks")
        _h = [None]
        mod.set_axon_ntff_profile_hook = lambda h: _h.__setitem__(0, h)
        mod.get_axon_ntff_profile_hook = lambda: _h[0]
        sys.modules["antenv.axon_hooks"] = mod
        antenv.axon_hooks = mod
        from trn_agent_boot.trn_boot import _ntff_profile_via_ctypes
        mod.set_axon_ntff_profile_hook(
            _ntff_profile_via_ctypes("/opt/axon/libaxon_pjrt.so"))
    except Exception:
        pass


def _prep_edges(edge_index):
    """Partition + window-pad edges. Returns (NW [T,2], Woff [T,2], Wtot,
    per-core idx arrays [128, Wtot*8] int16, slot arrays [128, SWtot] bf16,
    slot col offsets per tile)."""
    src = np.asarray(edge_index[0]).astype(np.int64)
    dst = np.asarray(edge_index[1]).astype(np.int64)
    loops = np.arange(N, dtype=np.int64)
    src = np.concatenate([src, loops])
    dst = np.concatenate([dst, loops])

    core = dst // NLOC
    ldst = dst % NLOC
    tilei = ldst // 128
    slot = ldst % 128
    # per-chunk global table layout: chunk q holds local rows
    # [CB[q], CB[q+1]) of every core, as its own (Shared) tensor.
    # run = chunk index; idx values are chunk-local (always < 32768).
    s_core = src // NLOC
    s_loc = src % NLOC
    q = np.searchsorted(np.asarray(CHUNKB[1:-1]), s_loc, side="right")
    cb = np.asarray(CHUNKB)[q]
    csz = (np.asarray(CHUNKB[1:]) - np.asarray(CHUNKB[:-1]))[q]
    srow = s_core * csz + (s_loc - cb)   # chunk-local row
    run = q

    nrun = len(CHUNKB) - 1
    cnt = np.zeros((NCORE, T, nrun), np.int64)
    np.add.at(cnt, (core, tilei, run), 1)
    NW = np.maximum(1, np.ceil(cnt.max(axis=0) / 128).astype(np.int64))  # [T,2]

    Woff = np.zeros((T, nrun), np.int64)
    w = 0
    for t in range(T):
        for r in range(nrun):
            Woff[t, r] = w
            w += NW[t, r]
    Wtot = int(w)

    # slot array column offsets: per (t) aligned to even
    SWoff = np.zeros((T, nrun), np.int64)
    sw = 0
    for t in range(T):
        for r in range(nrun):
            SWoff[t, r] = sw
            sw += NW[t, r] + (NW[t, r] & 1)
    SWtot = int(sw)

    order = np.lexsort((ldst, run, tilei, core))
    src_s = srow[order]
    core_s = core[order]
    tile_s = tilei[order]
    run_s = run[order]
    slot_s = slot[order]

    idx_arrs, slot_arrs = [], []
    # per (core,t,r) segment boundaries in the sorted arrays
    seg_key = ((core_s * T + tile_s) * nrun + run_s)
    bounds = np.searchsorted(seg_key, np.arange(NCORE * T * nrun + 1))
    for c in range(NCORE):
        idx16 = np.zeros((16, Wtot * 8), np.int16)
        slots = np.full((128, SWtot), 128.0, np.float32)
        for t in range(T):
            for r in range(nrun):
                k = (c * T + t) * nrun + r
                a, b = bounds[k], bounds[k + 1]
                n = b - a
                nw = int(NW[t, r])
                assert n <= nw * 128
                rows = src_s[a:b]
                sl = slot_s[a:b]
                j = np.arange(n)
                w0 = int(Woff[t, r])
                idx16[j % 16, w0 * 8 + j // 16] = rows.astype(np.int16)
                s0 = int(SWoff[t, r])
                slots[j % 128, s0 + j // 128] = sl
        idx_arrs.append(np.tile(idx16, (8, 1)))
        slot_arrs.append(slots.astype(BF))
    return NW, Woff, SWoff, Wtot, SWtot, idx_arrs, slot_arrs


def _build(NW, Woff, SWoff, Wtot, SWtot, ln_trivial, b1_zero, b2_zero):
    import concourse.bacc as bacc
    import concourse.mybir as mybir
    import concourse.tile as tile

    f32 = mybir.dt.float32
    bf = mybir.dt.bfloat16
    i16 = mybir.dt.int16
    AF = mybir.ActivationFunctionType
    ALU = mybir.AluOpType
    NWmax = int(NW.max())

    nc = bacc.Bacc("TRN2", target_bir_lowering=False, debug=False,
                   num_devices=NCORE, num_swdge_queues=NQ)

    x_in = nc.declare_dram_parameter("x", [NPAD, D], f32, isOutput=False)
    idx_in = nc.declare_dram_parameter("idx", [128, Wtot * 8], i16, isOutput=False)
    sl_in = nc.declare_dram_parameter("slots", [128, SWtot], bf, isOutput=False)
    w1_in = nc.declare_dram_parameter("w1e", [D, DH + 16], bf, isOutput=False)
    w2_in = nc.declare_dram_parameter("w2e", [DH, D + 2], bf, isOutput=False)
    io_in = nc.declare_dram_parameter("iotax", [128, 128 * NWmax], bf, isOutput=False)
    id_in = nc.declare_dram_parameter("ident", [128, 128], bf, isOutput=False)
    lnw_in = lnb_in = b1_in = b2_in = None
    if not ln_trivial:
        lnw_in = nc.declare_dram_parameter("lnw", [128, D], f32, isOutput=False)
        lnb_in = nc.declare_dram_parameter("lnb", [128, D], f32, isOutput=False)
    if not b1_zero:
        b1_in = nc.declare_dram_parameter("b1r", [128, DH], f32, isOutput=False)
    if not b2_zero:
        b2_in = nc.declare_dram_parameter("b2r", [128, D], f32, isOutput=False)
    out_ext = nc.declare_dram_parameter("out", [NPAD, D], f32, isOutput=True)

    nch = len(CHUNKB) - 1
    nrun = nch
    tab1_locs = [nc.dram_tensor(f"tab1_loc{q}", [CHUNKB[q + 1] - CHUNKB[q], ELEM1], bf)
                 for q in range(nch)]
    tab2_locs = [nc.dram_tensor(f"tab2_loc{q}", [CHUNKB[q + 1] - CHUNKB[q], ELEM2], bf)
                 for q in range(nch)]

    qrot = [0]

    def nextq():
        q = qrot[0]
        qrot[0] = (q + 1) % NQ
        return q

    with tile.TileContext(nc) as tc:
        with (
            tc.tile_pool(name="const", bufs=1) as cpool,
            tc.tile_pool(name="dram", bufs=1, space="DRAM") as dpool,
        ):
            tab1_fulls = [dpool.tile([NCORE * (CHUNKB[q + 1] - CHUNKB[q]), ELEM1],
                                     bf, addr_space="Shared", name=f"t1f{q}")
                          for q in range(nch)]
            tab2_fulls = [dpool.tile([NCORE * (CHUNKB[q + 1] - CHUNKB[q]), ELEM2],
                                     bf, addr_space="Shared", name=f"t2f{q}")
                          for q in range(nch)]

            # ---- constants to SBUF
            w1e = cpool.tile([128, 2, DH + 16], bf)
            nc.sync.dma_start(w1e[:], w1_in[:].rearrange("(k p) f -> p k f", p=128))
            w2e = cpool.tile([128, 4, D + 2], bf)
            nc.sync.dma_start(w2e[:], w2_in[:].rearrange("(k p) f -> p k f", p=128))
            iotax = cpool.tile([128, 128 * NWmax], bf)
            nc.sync.dma_start(iotax[:], io_in[:])
            ident = cpool.tile([128, 128], bf)
            nc.sync.dma_start(ident[:], id_in[:])
            slots_sb = cpool.tile([128, SWtot], bf)
            nc.sync.dma_start(slots_sb[:], sl_in[:])
            idx_sb = cpool.tile([128, Wtot * 8], i16)
            nc.sync.dma_start(idx_sb[:], idx_in[:])
            adst1 = cpool.tile([128, T * 8], bf)
            adst2 = cpool.tile([128, T], bf)
            if not ln_trivial:
                lnw_sb = cpool.tile([128, D], f32)
                nc.sync.dma_start(lnw_sb[:], lnw_in[:])
                lnb_sb = cpool.tile([128, D], f32)
                nc.sync.dma_start(lnb_sb[:], lnb_in[:])
            if not b1_zero:
                b1_sb = cpool.tile([128, DH], f32)
                nc.sync.dma_start(b1_sb[:], b1_in[:])
            if not b2_zero:
                b2_sb = cpool.tile([128, D], f32)
                nc.sync.dma_start(b2_sb[:], b2_in[:])

            iotax3 = iotax[:].rearrange("p (s w) -> p s w", w=NWmax)

            # ================= PHASE A: LN + dense1 + table1 =================
            pha = tc.tile_pool(name="phA", bufs=3)
            iop = pha.__enter__()
            wk_cm = tc.tile_pool(name="wkA", bufs=2)
            wkp = wk_cm.__enter__()
            sm_cm = tc.tile_pool(name="smA", bufs=3)
            smp = sm_cm.__enter__()
            psA_cm = tc.tile_pool(name="psA", bufs=2, space="PSUM")
            psA = psA_cm.__enter__()
            psT_cm = tc.tile_pool(name="psTA", bufs=2, space="PSUM")
            psT = psT_cm.__enter__()
            for t in range(T):
                xt = iop.tile([128, D], f32, tag="xt")
                nc.sync.dma_start(xt[:], x_in[t * 128:(t + 1) * 128, :])
                mean = smp.tile([128, 1], f32, tag="mean")
                nc.vector.reduce_sum(mean[:], xt[:], axis=mybir.AxisListType.X)
                nc.vector.tensor_scalar_mul(mean[:], mean[:], 1.0 / D)
                xc = wkp.tile([128, D], f32, tag="xc")
                nc.vector.tensor_scalar(xc[:], xt[:], mean[:], None, ALU.subtract)
                sq = smp.tile([128, 1], f32, tag="sq")
                sqj = wkp.tile([128, D], f32, tag="sqj")
                nc.scalar.activation(sqj[:], xc[:], AF.Square, accum_out=sq[:])
                nc.vector.tensor_scalar(sq[:], sq[:], 1.0 / D, EPS, ALU.mult, ALU.add)
                sd = smp.tile([128, 1], f32, tag="sd")
                nc.scalar.activation(sd[:], sq[:], AF.Sqrt)
                rstd = smp.tile([128, 1], f32, tag="rstd")
                nc.vector.reciprocal(rstd[:], sd[:])
                hbf = wkp.tile([128, D], bf, tag="hbf")
                if ln_trivial:
                    nc.scalar.activation(hbf[:], xc[:], AF.Copy, scale=rstd[:])
                else:
                    hf = wkp.tile([128, D], f32, tag="hf")
                    nc.scalar.activation(hf[:], xc[:], AF.Copy, scale=rstd[:])
                    nc.vector.tensor_mul(hf[:], hf[:], lnw_sb[:])
                    nc.vector.tensor_add(hbf[:], hf[:], lnb_sb[:])
                # transpose h -> [feat, node]
                hT = wkp.tile([128, 2, 128], bf, tag="hT")
                for k in range(2):
                    pst = psT.tile([128, 128], bf, tag="pstA")
                    nc.tensor.transpose(pst[:], hbf[:, k * 128:(k + 1) * 128], ident[:])
                    nc.scalar.copy(hT[:, k, :], pst[:])
                ps1 = psA.tile([128, DH], f32, tag="ps1")
                ps1b = psA.tile([128, 16], f32, tag="ps1b")
                for k in range(2):
                    nc.tensor.matmul(ps1[:], hT[:, k, :], w1e[:, k, 0:DH],
                                     start=(k == 0), stop=(k == 1))
                    nc.tensor.matmul(ps1b[:], hT[:, k, :],
                                     w1e[:, k, DH:DH + 16],
                                     start=(k == 0), stop=(k == 1))
                nc.scalar.copy(adst1[:, t * 8:(t + 1) * 8], ps1b[:, 8:16])
                tb = iop.tile([128, ELEM1], bf, tag="tb1")
                # ps1 is already c-major (W1e columns pre-permuted on host)
                nc.scalar.copy(tb[:, 0:DH], ps1[:])
                nc.scalar.copy(tb[:, DH:DH + 8], ps1b[:, 0:8])
                qch = next(i for i in range(nch) if t < CHUNKT[i + 1])
                r0 = t * 128 - CHUNKB[qch]
                nc.sync.dma_start(tab1_locs[qch][r0:r0 + 128, 0:DH + 8],
                                  tb[:, 0:DH + 8])
                if t == CHUNKT[qch + 1] - 1:
                    nc.gpsimd.collective_compute(
                        "AllGather", mybir.AluOpType.bypass,
                        replica_groups=[list(range(NCORE))],
                        ins=[tab1_locs[qch][:]],
                        outs=[tab1_fulls[qch].opt()],
                    )

            psT_cm.__exit__(None, None, None)
            psA_cm.__exit__(None, None, None)
            sm_cm.__exit__(None, None, None)
            wk_cm.__exit__(None, None, None)
            pha.__exit__(None, None, None)

            # ================= PHASE B: conv1 edges + dense2 =================
            NWT1 = int(NW.sum(axis=1).max())
            phb = tc.tile_pool(name="phB", bufs=3)
            iop = phb.__enter__()
            wk_cm = tc.tile_pool(name="wkB", bufs=2)
            wkp = wk_cm.__enter__()
            sm_cm = tc.tile_pool(name="smB", bufs=3)
            smp = sm_cm.__enter__()
            ga_cm = tc.tile_pool(name="gaB", bufs=3)
            gap = ga_cm.__enter__()
            st_cm = tc.tile_pool(name="stB", bufs=2)
            stp = st_cm.__enter__()
            psQ_cm = tc.tile_pool(name="psQ", bufs=2, space="PSUM")
            psQ = psQ_cm.__enter__()
            psZ_cm = tc.tile_pool(name="psZ", bufs=2, space="PSUM")
            psZ = psZ_cm.__enter__()
            psC_cm = tc.tile_pool(name="psC", bufs=1, space="PSUM")
            psC = psC_cm.__enter__()
            for t in range(T):
                nws = [int(NW[t, r]) for r in range(nrun)]
                nwt = sum(nws)
                gt = gap.tile([128, NWT1, ELEM1], bf, tag="gt1")
                for (rbase, w0g, w0l, nw) in _calls(t, nws, Woff):
                    nc.gpsimd.dma_gather(
                        gt[:, w0l:w0l + nw, :], tab1_fulls[rbase][:],
                        idx_sb[:, w0g * 8:(w0g + nw) * 8],
                        num_idxs=nw * 128, num_idxs_reg=nw * 128,
                        elem_size=ELEM1, queue_num=nextq(),
                    )
                P = stp.tile([128, 128 * NWT1], bf, tag="P1")
                Pv = P[:, :128 * nwt].rearrange("p (s w) -> p s w", w=nwt)
                wb = 0
                for r in range(nrun):
                    if nws[r] == 0:
                        continue
                    sl_r = slots_sb[:, int(SWoff[t, r]):int(SWoff[t, r]) + nws[r]]
                    nc.vector.tensor_tensor(
                        Pv[:, :, wb:wb + nws[r]],
                        sl_r.unsqueeze(1).broadcast_to([128, 128, nws[r]]),
                        iotax3[:, :, 0:nws[r]], ALU.is_equal)
                    wb += nws[r]
                Qp = psQ.tile([128, NWT1 * 128], bf, tag="Qp1")
                for w in range(nwt):
                    nc.tensor.transpose(Qp[:, w * 128:(w + 1) * 128],
                                        Pv[:, :, w], ident[:])
                Q = stp.tile([128, NWT1 * 128], bf, tag="Q1")
                nc.scalar.copy(Q[:, :nwt * 128], Qp[:, :nwt * 128])
                zb = psZ.tile([128, NWT1 * 8], f32, tag="zb1")
                for w in range(nwt):
                    nc.tensor.matmul(zb[:, w * 8:(w + 1) * 8],
                                     Q[:, w * 128:(w + 1) * 128],
                                     adst1[:, t * 8:(t + 1) * 8],
                                     start=True, stop=True)
                z = smp.tile([128, NWT1 * 8], f32, tag="z1")
                nc.vector.scalar_tensor_tensor(
                    z[:, :nwt * 8].rearrange("p (w d) -> p w d", d=8),
                    zb[:, :nwt * 8].rearrange("p (w d) -> p w d", d=8), 1.0,
                    gt[:, 0:nwt, DH:DH + 8],
                    ALU.mult, ALU.add)
                e2 = smp.tile([128, NWT1 * 8], f32, tag="e21")
                nc.scalar.activation(e2[:, :nwt * 8], z[:, :nwt * 8], AF.Exp, scale=NEG)
                e1 = smp.tile([128, NWT1 * 8], f32, tag="e11")
                nc.scalar.activation(e1[:, :nwt * 8], z[:, :nwt * 8], AF.Exp)
                stg = stp.tile([128, NWT1, 8 + DH], bf, tag="stg1")
                nc.vector.tensor_tensor(
                    stg[:, 0:nwt, 0:8],
                    e1[:, :nwt * 8].rearrange("p (w d) -> p w d", d=8),
                    e2[:, :nwt * 8].rearrange("p (w d) -> p w d", d=8),
                    ALU.max)
                # W'' = g (c-major) * p-bcast
                nc.vector.tensor_mul(
                    stg[:, 0:nwt, 8:8 + DH].rearrange("p w (c h) -> p w c h", h=8),
                    gt[:, 0:nwt, 0:DH].rearrange("p w (c h) -> p w c h", h=8),
                    stg[:, 0:nwt, 0:8].unsqueeze(2).broadcast_to([128, nwt, 64, 8]))
                oc = psC.tile([128, 1024], f32, tag="oc1")
                for w in range(nwt):
                    nc.tensor.matmul(oc[:, 0:8], Pv[:, :, w], stg[:, w, 0:8],
                                     start=(w == 0), stop=(w == nwt - 1))
                    nc.tensor.matmul(oc[:, 512:512 + DH], Pv[:, :, w], stg[:, w, 8:8 + DH],
                                     start=(w == 0), stop=(w == nwt - 1))
                den = smp.tile([128, 8], f32, tag="den1")
                nc.vector.tensor_scalar_max(den[:], oc[:, 0:8], 1e-30)
                rec = smp.tile([128, 8], f32, tag="rec1")
                nc.vector.reciprocal(rec[:], den[:])
                o1 = wkp.tile([128, DH], bf, tag="o1")
                nc.vector.tensor_tensor(
                    o1[:].rearrange("p (c h) -> p c h", h=8),
                    oc[:, 512:512 + DH].rearrange("p (c h) -> p c h", h=8),
                    rec[:].unsqueeze(1).broadcast_to([128, 64, 8]),
                    ALU.mult)
                if not b1_zero:
                    o1f = wkp.tile([128, DH], f32, tag="o1f")
                    nc.vector.tensor_add(o1f[:], o1[:], b1_sb[:])
                    o1 = o1f
                # ELU: h2 = relu(u) + exp(-relu(-u)) - 1
                pos = wkp.tile([128, DH], bf, tag="pos")
                nc.scalar.activation(pos[:], o1[:], AF.Relu)
                rneg = wkp.tile([128, DH], bf, tag="rneg")
                nc.scalar.activation(rneg[:], o1[:], AF.Relu, scale=-1.0)
                en = wkp.tile([128, DH], bf, tag="en")
                nc.scalar.activation(en[:], rneg[:], AF.Exp, scale=-1.0)
                h2 = wkp.tile([128, DH], bf, tag="h2")
                nc.vector.scalar_tensor_tensor(h2[:], pos[:], -1.0, en[:],
                                               ALU.add, ALU.add)
                # dense2
                hT2 = wkp.tile([128, 4, 128], bf, tag="hT2")
                pst = psZ.tile([128, 4, 128], bf, tag="zb1")
                for k in range(4):
                    nc.tensor.transpose(pst[:, k, :], h2[:, k * 128:(k + 1) * 128], ident[:])
                nc.scalar.copy(hT2[:], pst[:])
                ps2 = psQ.tile([128, D + 2], f32, tag="Qp1")
                for k in range(4):
                    nc.tensor.matmul(ps2[:], hT2[:, k, :], w2e[:, k, :],
                                     start=(k == 0), stop=(k == 3))
                nc.scalar.copy(adst2[:, t:t + 1], ps2[:, D + 1:D + 2])
                tb2 = iop.tile([128, ELEM2], bf, tag="tb2")
                nc.scalar.copy(tb2[:, 0:D + 1], ps2[:, 0:D + 1])
                qch = next(i for i in range(nch) if t < CHUNKT[i + 1])
                r0 = t * 128 - CHUNKB[qch]
                nc.sync.dma_start(tab2_locs[qch][r0:r0 + 128, 0:D + 1],
                                  tb2[:, 0:D + 1])
                if t == CHUNKT[qch + 1] - 1:
                    nc.gpsimd.collective_compute(
                        "AllGather", mybir.AluOpType.bypass,
                        replica_groups=[list(range(NCORE))],
                        ins=[tab2_locs[qch][:]],
                        outs=[tab2_fulls[qch].opt()],
                    )

            psC_cm.__exit__(None, None, None)
            psZ_cm.__exit__(None, None, None)
            psQ_cm.__exit__(None, None, None)
            st_cm.__exit__(None, None, None)
            ga_cm.__exit__(None, None, None)
            sm_cm.__exit__(None, None, None)
            wk_cm.__exit__(None, None, None)
            phb.__exit__(None, None, None)

            # ================= PHASE C: conv2 edges =================
            phc = tc.tile_pool(name="phC", bufs=3)
            iop = phc.__enter__()
            sm_cm = tc.tile_pool(name="smC", bufs=3)
            smp = sm_cm.__enter__()
            ga_cm = tc.tile_pool(name="gaC", bufs=3)
            gap = ga_cm.__enter__()
            st_cm = tc.tile_pool(name="stC", bufs=2)
            stp = st_cm.__enter__()
            psQ_cm = tc.tile_pool(name="psQC", bufs=2, space="PSUM")
            psQ = psQ_cm.__enter__()
            psZ_cm = tc.tile_pool(name="psZC", bufs=2, space="PSUM")
            psZ = psZ_cm.__enter__()
            psC_cm = tc.tile_pool(name="psCC", bufs=1, space="PSUM")
            psC = psC_cm.__enter__()
            for t in range(T):
                nws = [int(NW[t, r]) for r in range(nrun)]
                nwt = sum(nws)
                gt = gap.tile([128, NWT1, ELEM2], bf, tag="gt2")
                for (rbase, w0g, w0l, nw) in _calls(t, nws, Woff):
                    nc.gpsimd.dma_gather(
                        gt[:, w0l:w0l + nw, :], tab2_fulls[rbase][:],
                        idx_sb[:, w0g * 8:(w0g + nw) * 8],
                        num_idxs=nw * 128, num_idxs_reg=nw * 128,
                        elem_size=ELEM2, queue_num=nextq(),
                    )
                P = stp.tile([128, 128 * NWT1], bf, tag="P1")
                Pv = P[:, :128 * nwt].rearrange("p (s w) -> p s w", w=nwt)
                wb = 0
                for r in range(nrun):
                    if nws[r] == 0:
                        continue
                    sl_r = slots_sb[:, int(SWoff[t, r]):int(SWoff[t, r]) + nws[r]]
                    nc.vector.tensor_tensor(
                        Pv[:, :, wb:wb + nws[r]],
                        sl_r.unsqueeze(1).broadcast_to([128, 128, nws[r]]),
                        iotax3[:, :, 0:nws[r]], ALU.is_equal)
                    wb += nws[r]
                Qp = psQ.tile([128, NWT1 * 128], bf, tag="Qp2")
                for w in range(nwt):
                    nc.tensor.transpose(Qp[:, w * 128:(w + 1) * 128],
                                        Pv[:, :, w], ident[:])
                Q = stp.tile([128, NWT1 * 128], bf, tag="Q2")
                nc.scalar.copy(Q[:, :nwt * 128], Qp[:, :nwt * 128])
                zb = psZ.tile([128, NWT1], f32, tag="zb2")
                for w in range(nwt):
                    nc.tensor.matmul(zb[:, w:w + 1],
                                     Q[:, w * 128:(w + 1) * 128],
                                     adst2[:, t:t + 1],
                                     start=True, stop=True)
                z = smp.tile([128, NWT1], f32, tag="z2")
                nc.vector.scalar_tensor_tensor(
                    z[:, :nwt].rearrange("p (w d) -> p w d", d=1),
                    zb[:, :nwt].rearrange("p (w d) -> p w d", d=1), 1.0,
                    gt[:, 0:nwt, D:D + 1],
                    ALU.mult, ALU.add)
                e2 = smp.tile([128, NWT1], f32, tag="e22")
                nc.scalar.activation(e2[:, :nwt], z[:, :nwt], AF.Exp, scale=NEG)
                e1 = smp.tile([128, NWT1], f32, tag="e12")
                nc.scalar.activation(e1[:, :nwt], z[:, :nwt], AF.Exp)
                p2 = smp.tile([128, NWT1], f32, tag="p2")
                nc.vector.tensor_max(p2[:, :nwt], e1[:, :nwt], e2[:, :nwt])
                stg = stp.tile([128, NWT1, 8 + D], bf, tag="stg2")
                # replicate p2 into 8 pseudo-head cols
                nc.scalar.copy(
                    stg[:, 0:nwt, 0:8],
                    p2[:, :nwt].unsqueeze(2).broadcast_to([128, nwt, 8]))
                nc.vector.tensor_mul(
                    stg[:, 0:nwt, 8:8 + D].rearrange("p w (c h) -> p w c h", h=8),
                    gt[:, 0:nwt, 0:D].rearrange("p w (c h) -> p w c h", h=8),
                    stg[:, 0:nwt, 0:8].unsqueeze(2).broadcast_to([128, nwt, 32, 8]))
                oc2 = psC.tile([128, 1024], f32, tag="oc2")
                for w in range(nwt):
                    nc.tensor.matmul(oc2[:, 0:8], Pv[:, :, w], stg[:, w, 0:8],
                                     start=(w == 0), stop=(w == nwt - 1))
                    nc.tensor.matmul(oc2[:, 512:512 + D], Pv[:, :, w], stg[:, w, 8:8 + D],
                                     start=(w == 0), stop=(w == nwt - 1))
                den = smp.tile([128, 1], f32, tag="den2")
                nc.vector.tensor_scalar_max(den[:], oc2[:, 0:1], 1e-30)
                rec = smp.tile([128, 1], f32, tag="rec2")
                nc.vector.reciprocal(rec[:], den[:])
                outt = iop.tile([128, D], f32, tag="outt")
                # un-permute c-major -> natural: out[h*32+c] = oc[8 + c*8+h]
                nc.vector.tensor_scalar(
                    outt[:].rearrange("p (h c) -> p h c", c=32),
                    oc2[:, 512:512 + D].rearrange("p (c h) -> p c h", h=8).transpose([0, 2, 1]),
                    rec[:], None, ALU.mult)
                if not b2_zero:
                    nc.vector.tensor_add(outt[:], outt[:], b2_sb[:])
                nc.sync.dma_start(out_ext[t * 128:(t + 1) * 128, :], outt[:])
            psC_cm.__exit__(None, None, None)
            psZ_cm.__exit__(None, None, None)
            psQ_cm.__exit__(None, None, None)
            st_cm.__exit__(None, None, None)
            ga_cm.__exit__(None, None, None)
            sm_cm.__exit__(None, None, None)
            phc.__exit__(None, None, None)

    nc.compile()
    return nc


def _calls(t, nws, Woff):
    """Gather call plan for tile t: (run_base, global_w0, local_w0, nw)."""
    out = []
    lbase = 0
    for r, nwr in enumerate(nws):
        w0 = int(Woff[t, r])
        done = 0
        while done < nwr:
            nw = min(4, nwr - done)
            out.append((r, w0 + done, lbase + done, nw))
            done += nw
        lbase += nwr
    return out


def _host_prep(inputs):
    edge_index = np.asarray(inputs["edge_index"])
    x = np.asarray(inputs["x"], np.float32)
    ln_w = np.asarray(inputs["ln_w"], np.float32)
    ln_b = np.asarray(inputs["ln_b"], np.float32)
    W1 = np.asarray(inputs["W1"], np.float32)
    a_s1 = np.asarray(inputs["att_src1"], np.float32)
    a_d1 = np.asarray(inputs["att_dst1"], np.float32)
    b1 = np.asarray(inputs["b1"], np.float32)
    W2 = np.asarray(inputs["W2"], np.float32)
    a_s2 = np.asarray(inputs["att_src2"], np.float32)
    a_d2 = np.asarray(inputs["att_dst2"], np.float32)
    b2 = np.asarray(inputs["b2"], np.float32)

    NW, Woff, SWoff, Wtot, SWtot, idx_arrs, slot_arrs = _prep_edges(edge_index)
    NWmax = int(NW.max())

    # W1_ext: c-major permuted cols + attention folds
    perm1 = np.empty(DH, np.int64)
    for h in range(H1):
        for c in range(C1):
            perm1[c * 8 + h] = h * C1 + c
    W1p = W1[:, perm1]
    wsrc1 = np.stack([W1[:, h * C1:(h + 1) * C1] @ a_s1[h] for h in range(H1)], 1)
    wdst1 = np.stack([W1[:, h * C1:(h + 1) * C1] @ a_d1[h] for h in range(H1)], 1)
    w1e = np.concatenate([W1p, wsrc1, wdst1], axis=1).astype(BF)  # [256, 528]

    # W2_ext: rows permuted to h2's c-major layout; cols permuted to
    # pseudo-head c-major (8 groups of 32); + attention folds
    W2r = W2[perm1, :]
    perm2 = np.empty(D, np.int64)
    for h in range(8):
        for c in range(32):
            perm2[c * 8 + h] = h * 32 + c
    W2p = W2r[:, perm2]
    wsrc2 = W2r @ a_s2[0]
    wdst2 = W2r @ a_d2[0]
    w2e = np.concatenate([W2p, wsrc2[:, None], wdst2[:, None]], axis=1).astype(BF)

    iotax = np.zeros((128, 128 * NWmax), np.float32)
    for s in range(128):
        iotax[:, s * NWmax:(s + 1) * NWmax] = s
    iotax = iotax.astype(BF)
    identm = np.eye(128).astype(BF)

    ln_trivial = bool(np.all(ln_w == 1.0) and np.all(ln_b == 0.0))
    b1_zero = bool(np.all(b1 == 0.0))
    b2_zero = bool(np.all(b2 == 0.0))

    in_maps = []
    for c in range(NCORE):
        xp = np.zeros((NPAD, D), np.float32)
        xp[:NLOC] = x[c * NLOC:(c + 1) * NLOC]
        m = {
            "x": xp, "idx": idx_arrs[c], "slots": slot_arrs[c],
            "w1e": w1e, "w2e": w2e, "iotax": iotax, "ident": identm,
        }
        if not ln_trivial:
            m["lnw"] = np.tile(ln_w[None, :], (128, 1)).astype(np.float32)
            m["lnb"] = np.tile(ln_b[None, :], (128, 1)).astype(np.float32)
        if not b1_zero:
            m["b1r"] = np.tile(b1[perm1][None, :], (128, 1)).astype(np.float32)
        if not b2_zero:
            m["b2r"] = np.tile(b2[None, :], (128, 1)).astype(np.float32)
        in_maps.append(m)
    meta = (NW, Woff, SWoff, Wtot, SWtot, ln_trivial, b1_zero, b2_zero)
    return meta, in_maps


def kernel(**inputs):
    _install_ntff_hook()
    from concourse.bass_utils import run_bass_kernel_spmd

    meta, in_maps = _host_prep(inputs)
    NW, Woff, SWoff, Wtot, SWtot, ln_trivial, b1_zero, b2_zero = meta
    key = (Wtot, SWtot, ln_trivial, b1_zero, b2_zero, NW.tobytes())
    if key not in _cache:
        _cache[key] = _build(NW, Woff, SWoff, Wtot, SWtot,
                             ln_trivial, b1_zero, b2_zero)
    nc = _cache[key]

    trace = bool(int(__import__("os").environ.get("KERNEL_TRACE", "0")))
    res = run_bass_kernel_spmd(nc, in_maps, core_ids=list(range(NCORE)),
                               trace=trace)
    kernel.last_exec_time_ns = res.exec_time_ns
    out = np.concatenate([res.results[c]["out"][:NLOC] for c in range(NCORE)], 0)
    return out.astype(np.float32)


kernel.last_exec_time_ns = None



# revision 6
# speedup vs baseline: 1.1475x; 1.1475x over previous
"""Distributed 2-layer GAT (nn_AlignHead) on 8 TRN2 NeuronCores.

Strategy: shard nodes (dst) contiguously across 8 cores. Per core:
  Phase A: LayerNorm + h@W1_ext dense matmul -> per-node table rows
           [g1 (c-major, 512) | a_src1 (8) | pad] bf16; chunked AllGather
           (uneven chunks: big chunk overlaps compute, small tail chunk).
  Phase B: per dst-tile (128 dsts): dma_gather edge src rows, segment
           softmax via indicator matmuls. P[e,s] built on DVE (is_equal);
           P^T built DIRECTLY on DVE from host staircase bounds (edges are
           slot-sorted per window => P^T rows are column ranges:
           (iota>=start)*(iota<end), 2 DVE ops). a_dst broadcast via
           P^T-matmul, p = max(exp(z), exp(0.2 z)), weighted aggregation +
           denominator fused in ONE 520-col matmul per window, normalize,
           ELU -> h2, dense h2@W2_ext -> table2; chunked AllGather.
  Phase C: conv2 edge phase (1 head): attention scalar folded INTO P
           (per-partition scale), single 258-col matmul per window with a
           ones-column denominator; normalize -> output rows.

Self-contained: hardcodes the problem shapes; compiles on first call.
"""
import sys
import types

import numpy as np
import ml_dtypes

# ---------------------------------------------------------------- constants
NCORE = 8
N = 50000
E = 500000
D = 256
H1, C1 = 8, 64
DH = 512            # H1*C1
NEG = 0.2
EPS = 1e-5
NLOC = 6250         # nodes per core
NPAD = 6272         # 49*128
T = 49              # dst tiles per core
ROWS = NPAD * NCORE  # 50176 global (padded) table rows
LO = 32768          # int16 gather row limit
ELEM1 = 640         # bf16 elems per conv1 table row (1280 B)
ELEM2 = 384         # bf16 elems per conv2 table row (768 B)
NQ = 4              # swdge queues
# AG chunks double as the int16 gather-range split: chunk q's global table
# [8*(CB[q+1]-CB[q]) rows] fits int16 indexing entirely. Uneven on purpose:
# the big chunk's AllGather overlaps compute; only the small tail blocks.
CHUNKB = [0, 3968, 6272]   # AG chunk boundaries (local rows)  8*3968=31744<32768
CHUNKT = [0, 31, 49]       # tile boundaries per chunk
BF = ml_dtypes.bfloat16
GCAP = 8            # max windows per dma_gather call

_cache = {}


def _install_ntff_hook():
    if "antenv.axon_hooks" in sys.modules:
        return
    try:
        import antenv
        mod = types.ModuleType("antenv.axon_hooks")
        _h = [None]
        mod.set_axon_ntff_profile_hook = lambda h: _h.__setitem__(0, h)
        mod.get_axon_ntff_profile_hook = lambda: _h[0]
        sys.modules["antenv.axon_hooks"] = mod
        antenv.axon_hooks = mod
        from trn_agent_boot.trn_boot import _ntff_profile_via_ctypes
        mod.set_axon_ntff_profile_hook(
            _ntff_profile_via_ctypes("/opt/axon/libaxon_pjrt.so"))
    except Exception:
        pass


def _prep_edges(edge_index):
    """Partition + window-pad edges. Returns (NW [T,nrun], Woff, SWoff, Wtot,
    SWtot, per-core idx arrays [128, Wtot*8] int16, slot arrays [128, SWtot]
    bf16, staircase start/end arrays [128, SWtot] bf16)."""
    src = np.asarray(edge_index[0]).astype(np.int64)
    dst = np.asarray(edge_index[1]).astype(np.int64)
    loops = np.arange(N, dtype=np.int64)
    src = np.concatenate([src, loops])
    dst = np.concatenate([dst, loops])

    core = dst // NLOC
    ldst = dst % NLOC
    tilei = ldst // 128
    slot = ldst % 128
    # per-chunk global table layout: chunk q holds local rows
    # [CB[q], CB[q+1]) of every core, as its own (Shared) tensor.
    # run = chunk index; idx values are chunk-local (always < 32768).
    s_core = src // NLOC
    s_loc = src % NLOC
    q = np.searchsorted(np.asarray(CHUNKB[1:-1]), s_loc, side="right")
    cb = np.asarray(CHUNKB)[q]
    csz = (np.asarray(CHUNKB[1:]) - np.asarray(CHUNKB[:-1]))[q]
    srow = s_core * csz + (s_loc - cb)   # chunk-local row
    run = q

    nrun = len(CHUNKB) - 1
    cnt = np.zeros((NCORE, T, nrun), np.int64)
    np.add.at(cnt, (core, tilei, run), 1)
    NW = np.maximum(1, np.ceil(cnt.max(axis=0) / 128).astype(np.int64))  # [T,nrun]

    Woff = np.zeros((T, nrun), np.int64)
    w = 0
    for t in range(T):
        for r in range(nrun):
            Woff[t, r] = w
            w += NW[t, r]
    Wtot = int(w)

    # slot array column offsets: per (t) aligned to even
    SWoff = np.zeros((T, nrun), np.int64)
    sw = 0
    for t in range(T):
        for r in range(nrun):
            SWoff[t, r] = sw
            sw += NW[t, r] + (NW[t, r] & 1)
    SWtot = int(sw)

    order = np.lexsort((ldst, run, tilei, core))
    src_s = srow[order]
    core_s = core[order]
    tile_s = tilei[order]
    run_s = run[order]
    slot_s = slot[order]

    idx_arrs, slot_arrs, start_arrs, end_arrs = [], [], [], []
    # per (core,t,r) segment boundaries in the sorted arrays
    seg_key = ((core_s * T + tile_s) * nrun + run_s)
    bounds = np.searchsorted(seg_key, np.arange(NCORE * T * nrun + 1))
    srange = np.arange(128)
    for c in range(NCORE):
        idx16 = np.zeros((16, Wtot * 8), np.int16)
        slots = np.full((128, SWtot), 128.0, np.float32)
        startv = np.zeros((128, SWtot), np.float32)
        endv = np.zeros((128, SWtot), np.float32)
        for t in range(T):
            for r in range(nrun):
                k = (c * T + t) * nrun + r
                a, b = bounds[k], bounds[k + 1]
                n = b - a
                nw = int(NW[t, r])
                assert n <= nw * 128
                rows = src_s[a:b]
                sl = slot_s[a:b]
                j = np.arange(n)
                w0 = int(Woff[t, r])
                idx16[j % 16, w0 * 8 + j // 16] = rows.astype(np.int16)
                s0 = int(SWoff[t, r])
                slots[j % 128, s0 + j // 128] = sl
                # staircase bounds for direct P^T build: within window w the
                # edges are slot-sorted, so P^T[s,:] is the col range
                # [start_s, end_s).
                for w in range(nw):
                    wsl = sl[w * 128:(w + 1) * 128]
                    startv[:, s0 + w] = np.searchsorted(wsl, srange, "left")
                    endv[:, s0 + w] = np.searchsorted(wsl, srange, "right")
        idx_arrs.append(np.tile(idx16, (8, 1)))
        slot_arrs.append(slots.astype(BF))
        start_arrs.append(startv.astype(BF))
        end_arrs.append(endv.astype(BF))
    return NW, Woff, SWoff, Wtot, SWtot, idx_arrs, slot_arrs, start_arrs, end_arrs


def _build(NW, Woff, SWoff, Wtot, SWtot, ln_trivial, b1_zero, b2_zero):
    import concourse.bacc as bacc
    import concourse.mybir as mybir
    import concourse.tile as tile

    f32 = mybir.dt.float32
    bf = mybir.dt.bfloat16
    i16 = mybir.dt.int16
    AF = mybir.ActivationFunctionType
    ALU = mybir.AluOpType
    NWmax = int(NW.max())
    NWT1 = int(NW.sum(axis=1).max())

    nc = bacc.Bacc("TRN2", target_bir_lowering=False, debug=False,
                   num_devices=NCORE, num_swdge_queues=NQ)

    x_in = nc.declare_dram_parameter("x", [NPAD, D], f32, isOutput=False)
    idx_in = nc.declare_dram_parameter("idx", [128, Wtot * 8], i16, isOutput=False)
    sl_in = nc.declare_dram_parameter("slots", [128, SWtot], bf, isOutput=False)
    st_in = nc.declare_dram_parameter("startv", [128, SWtot], bf, isOutput=False)
    en_in = nc.declare_dram_parameter("endv", [128, SWtot], bf, isOutput=False)
    w1_in = nc.declare_dram_parameter("w1e", [D, DH + 16], bf, isOutput=False)
    w2_in = nc.declare_dram_parameter("w2e", [DH, D + 2], bf, isOutput=False)
    io_in = nc.declare_dram_parameter("iotax", [128, 128 * NWmax], bf, isOutput=False)
    ie_in = nc.declare_dram_parameter("iotae", [128, 128], bf, isOutput=False)
    id_in = nc.declare_dram_parameter("ident", [128, 128], bf, isOutput=False)
    lnw_in = lnb_in = b1_in = b2_in = None
    if not ln_trivial:
        lnw_in = nc.declare_dram_parameter("lnw", [128, D], f32, isOutput=False)
        lnb_in = nc.declare_dram_parameter("lnb", [128, D], f32, isOutput=False)
    if not b1_zero:
        b1_in = nc.declare_dram_parameter("b1r", [128, DH], f32, isOutput=False)
    if not b2_zero:
        b2_in = nc.declare_dram_parameter("b2r", [128, D], f32, isOutput=False)
    out_ext = nc.declare_dram_parameter("out", [NPAD, D], f32, isOutput=True)

    nch = len(CHUNKB) - 1
    nrun = nch
    tab1_locs = [nc.dram_tensor(f"tab1_loc{q}", [CHUNKB[q + 1] - CHUNKB[q], ELEM1], bf)
                 for q in range(nch)]
    tab2_locs = [nc.dram_tensor(f"tab2_loc{q}", [CHUNKB[q + 1] - CHUNKB[q], ELEM2], bf)
                 for q in range(nch)]

    qrot = [0]

    def nextq():
        q = qrot[0]
        qrot[0] = (q + 1) % NQ
        return q

    with tile.TileContext(nc) as tc:
        with (
            tc.tile_pool(name="const", bufs=1) as cpool,
            tc.tile_pool(name="dram", bufs=1, space="DRAM") as dpool,
        ):
            tab1_fulls = [dpool.tile([NCORE * (CHUNKB[q + 1] - CHUNKB[q]), ELEM1],
                                     bf, addr_space="Shared", name=f"t1f{q}")
                          for q in range(nch)]
            tab2_fulls = [dpool.tile([NCORE * (CHUNKB[q + 1] - CHUNKB[q]), ELEM2],
                                     bf, addr_space="Shared", name=f"t2f{q}")
                          for q in range(nch)]

            # ---- constants to SBUF
            w1e = cpool.tile([128, 2, DH + 16], bf)
            nc.sync.dma_start(w1e[:], w1_in[:].rearrange("(k p) f -> p k f", p=128))
            w2e = cpool.tile([128, 4, D + 2], bf)
            nc.sync.dma_start(w2e[:], w2_in[:].rearrange("(k p) f -> p k f", p=128))
            iotax = cpool.tile([128, 128 * NWmax], bf)
            nc.sync.dma_start(iotax[:], io_in[:])
            iotae = cpool.tile([128, 128], bf)
            nc.sync.dma_start(iotae[:], ie_in[:])
            ident = cpool.tile([128, 128], bf)
            nc.sync.dma_start(ident[:], id_in[:])
            slots_sb = cpool.tile([128, SWtot], bf)
            nc.sync.dma_start(slots_sb[:], sl_in[:])
            start_sb = cpool.tile([128, SWtot], bf)
            nc.sync.dma_start(start_sb[:], st_in[:])
            end_sb = cpool.tile([128, SWtot], bf)
            nc.sync.dma_start(end_sb[:], en_in[:])
            idx_sb = cpool.tile([128, Wtot * 8], i16)
            nc.sync.dma_start(idx_sb[:], idx_in[:])
            adst1 = cpool.tile([128, T * 8], bf)
            adst2 = cpool.tile([128, T], bf)
            if not ln_trivial:
                lnw_sb = cpool.tile([128, D], f32)
                nc.sync.dma_start(lnw_sb[:], lnw_in[:])
                lnb_sb = cpool.tile([128, D], f32)
                nc.sync.dma_start(lnb_sb[:], lnb_in[:])
            if not b1_zero:
                b1_sb = cpool.tile([128, DH], f32)
                nc.sync.dma_start(b1_sb[:], b1_in[:])
            if not b2_zero:
                b2_sb = cpool.tile([128, D], f32)
                nc.sync.dma_start(b2_sb[:], b2_in[:])

            iotax3 = iotax[:].rearrange("p (s w) -> p s w", w=NWmax)

            # ================= PHASE A: LN + dense1 + table1 =================
            pha = tc.tile_pool(name="phA", bufs=3)
            iop = pha.__enter__()
            wk_cm = tc.tile_pool(name="wkA", bufs=2)
            wkp = wk_cm.__enter__()
            sm_cm = tc.tile_pool(name="smA", bufs=3)
            smp = sm_cm.__enter__()
            psA_cm = tc.tile_pool(name="psA", bufs=2, space="PSUM")
            psA = psA_cm.__enter__()
            psT_cm = tc.tile_pool(name="psTA", bufs=2, space="PSUM")
            psT = psT_cm.__enter__()
            for t in range(T):
                xt = iop.tile([128, D], f32, tag="xt")
                nc.sync.dma_start(xt[:], x_in[t * 128:(t + 1) * 128, :])
                mean = smp.tile([128, 1], f32, tag="mean")
                nc.vector.reduce_sum(mean[:], xt[:], axis=mybir.AxisListType.X)
                nc.vector.tensor_scalar_mul(mean[:], mean[:], 1.0 / D)
                xc = wkp.tile([128, D], f32, tag="xc")
                nc.vector.tensor_scalar(xc[:], xt[:], mean[:], None, ALU.subtract)
                sq = smp.tile([128, 1], f32, tag="sq")
                sqj = wkp.tile([128, D], f32, tag="sqj")
                nc.scalar.activation(sqj[:], xc[:], AF.Square, accum_out=sq[:])
                nc.vector.tensor_scalar(sq[:], sq[:], 1.0 / D, EPS, ALU.mult, ALU.add)
                sd = smp.tile([128, 1], f32, tag="sd")
                nc.scalar.activation(sd[:], sq[:], AF.Sqrt)
                rstd = smp.tile([128, 1], f32, tag="rstd")
                nc.vector.reciprocal(rstd[:], sd[:])
                hbf = wkp.tile([128, D], bf, tag="hbf")
                if ln_trivial:
                    nc.scalar.activation(hbf[:], xc[:], AF.Copy, scale=rstd[:])
                else:
                    hf = wkp.tile([128, D], f32, tag="hf")
                    nc.scalar.activation(hf[:], xc[:], AF.Copy, scale=rstd[:])
                    nc.vector.tensor_mul(hf[:], hf[:], lnw_sb[:])
                    nc.vector.tensor_add(hbf[:], hf[:], lnb_sb[:])
                # transpose h -> [feat, node]
                hT = wkp.tile([128, 2, 128], bf, tag="hT")
                for k in range(2):
                    pst = psT.tile([128, 128], bf, tag="pstA")
                    nc.tensor.transpose(pst[:], hbf[:, k * 128:(k + 1) * 128], ident[:])
                    nc.scalar.copy(hT[:, k, :], pst[:])
                ps1 = psA.tile([128, DH], f32, tag="ps1")
                ps1b = psA.tile([128, 16], f32, tag="ps1b")
                for k in range(2):
                    nc.tensor.matmul(ps1[:], hT[:, k, :], w1e[:, k, 0:DH],
                                     start=(k == 0), stop=(k == 1))
                    nc.tensor.matmul(ps1b[:], hT[:, k, :],
                                     w1e[:, k, DH:DH + 16],
                                     start=(k == 0), stop=(k == 1))
                nc.scalar.copy(adst1[:, t * 8:(t + 1) * 8], ps1b[:, 8:16])
                tb = iop.tile([128, ELEM1], bf, tag="tb1")
                # ps1 is already c-major (W1e columns pre-permuted on host)
                nc.scalar.copy(tb[:, 0:DH], ps1[:])
                nc.scalar.copy(tb[:, DH:DH + 8], ps1b[:, 0:8])
                qch = next(i for i in range(nch) if t < CHUNKT[i + 1])
                r0 = t * 128 - CHUNKB[qch]
                nc.sync.dma_start(tab1_locs[qch][r0:r0 + 128, 0:DH + 8],
                                  tb[:, 0:DH + 8])
                if t == CHUNKT[qch + 1] - 1:
                    nc.gpsimd.collective_compute(
                        "AllGather", mybir.AluOpType.bypass,
                        replica_groups=[list(range(NCORE))],
                        ins=[tab1_locs[qch][:]],
                        outs=[tab1_fulls[qch].opt()],
                    )

            psT_cm.__exit__(None, None, None)
            psA_cm.__exit__(None, None, None)
            sm_cm.__exit__(None, None, None)
            wk_cm.__exit__(None, None, None)
            pha.__exit__(None, None, None)

            # ================= PHASE B: conv1 edges + dense2 =================
            phb = tc.tile_pool(name="phB", bufs=3)
            iop = phb.__enter__()
            wk_cm = tc.tile_pool(name="wkB", bufs=2)
            wkp = wk_cm.__enter__()
            sm_cm = tc.tile_pool(name="smB", bufs=3)
            smp = sm_cm.__enter__()
            ga_cm = tc.tile_pool(name="gaB", bufs=3)
            gap = ga_cm.__enter__()
            st_cm = tc.tile_pool(name="stB", bufs=2)
            stp = st_cm.__enter__()
            psZ_cm = tc.tile_pool(name="psZ", bufs=2, space="PSUM")
            psZ = psZ_cm.__enter__()
            psD_cm = tc.tile_pool(name="psD", bufs=2, space="PSUM")
            psD = psD_cm.__enter__()
            psC_cm = tc.tile_pool(name="psC", bufs=2, space="PSUM")
            psC = psC_cm.__enter__()
            for t in range(T):
                nws = [int(NW[t, r]) for r in range(nrun)]
                nwt = sum(nws)
                gt = gap.tile([128, NWT1, ELEM1], bf, tag="gt1")
                for (rbase, w0g, w0l, nw) in _calls(t, nws, Woff):
                    nc.gpsimd.dma_gather(
                        gt[:, w0l:w0l + nw, :], tab1_fulls[rbase][:],
                        idx_sb[:, w0g * 8:(w0g + nw) * 8],
                        num_idxs=nw * 128, num_idxs_reg=nw * 128,
                        elem_size=ELEM1, queue_num=nextq(),
                    )
                P = stp.tile([128, 128 * NWT1], bf, tag="P1")
                Pv = P[:, :128 * nwt].rearrange("p (s w) -> p s w", w=nwt)
                wb = 0
                for r in range(nrun):
                    if nws[r] == 0:
                        continue
                    sl_r = slots_sb[:, int(SWoff[t, r]):int(SWoff[t, r]) + nws[r]]
                    nc.vector.tensor_tensor(
                        Pv[:, :, wb:wb + nws[r]],
                        sl_r.unsqueeze(1).broadcast_to([128, 128, nws[r]]),
                        iotax3[:, :, 0:nws[r]], ALU.is_equal)
                    wb += nws[r]
                # P^T built directly on DVE from staircase bounds:
                # Pt[s, w, j] = (j >= start[s,w]) - (j >= end[s,w])
                Pt = stp.tile([128, NWT1, 128], bf, tag="Pt1")
                m1 = stp.tile([128, NWT1, 128], bf, tag="m11")
                m2 = stp.tile([128, NWT1, 128], bf, tag="m21")
                wb = 0
                for r in range(nrun):
                    s0 = int(SWoff[t, r])
                    nw = nws[r]
                    iob = iotae[:].unsqueeze(1).broadcast_to([128, nw, 128])
                    nc.vector.tensor_tensor(
                        m1[:, wb:wb + nw, :], iob,
                        start_sb[:, s0:s0 + nw].unsqueeze(2)
                        .broadcast_to([128, nw, 128]), ALU.is_ge)
                    nc.vector.tensor_tensor(
                        m2[:, wb:wb + nw, :], iob,
                        end_sb[:, s0:s0 + nw].unsqueeze(2)
                        .broadcast_to([128, nw, 128]), ALU.is_ge)
                    nc.vector.tensor_tensor(
                        Pt[:, wb:wb + nw, :], m1[:, wb:wb + nw, :],
                        m2[:, wb:wb + nw, :], ALU.subtract)
                    wb += nws[r]
                zb = psZ.tile([128, NWT1 * 8], f32, tag="zb1")
                for w in range(nwt):
                    nc.tensor.matmul(zb[:, w * 8:(w + 1) * 8],
                                     Pt[:, w, :],
                                     adst1[:, t * 8:(t + 1) * 8],
                                     start=True, stop=True)
                z = smp.tile([128, NWT1 * 8], f32, tag="z1")
                nc.vector.scalar_tensor_tensor(
                    z[:, :nwt * 8].rearrange("p (w d) -> p w d", d=8),
                    zb[:, :nwt * 8].rearrange("p (w d) -> p w d", d=8), 1.0,
                    gt[:, 0:nwt, DH:DH + 8],
                    ALU.mult, ALU.add)
                e2 = smp.tile([128, NWT1 * 8], f32, tag="e21")
                nc.scalar.activation(e2[:, :nwt * 8], z[:, :nwt * 8], AF.Exp, scale=NEG)
                e1 = smp.tile([128, NWT1 * 8], f32, tag="e11")
                nc.scalar.activation(e1[:, :nwt * 8], z[:, :nwt * 8], AF.Exp)
                stg = stp.tile([128, NWT1, 8 + DH], bf, tag="stg1")
                nc.vector.tensor_tensor(
                    stg[:, 0:nwt, 0:8],
                    e1[:, :nwt * 8].rearrange("p (w d) -> p w d", d=8),
                    e2[:, :nwt * 8].rearrange("p (w d) -> p w d", d=8),
                    ALU.max)
                # W'' = g (c-major) * p-bcast
                nc.vector.tensor_mul(
                    stg[:, 0:nwt, 8:8 + DH].rearrange("p w (c h) -> p w c h", h=8),
                    gt[:, 0:nwt, 0:DH].rearrange("p w (c h) -> p w c h", h=8),
                    stg[:, 0:nwt, 0:8].unsqueeze(2).broadcast_to([128, nwt, 64, 8]))
                # denominator (bank 0, cols 0:8) + numerator (bank 1): one
                # matmul may not cross a PSUM bank boundary
                oc = psC.tile([128, 1024], f32, tag="oc1")
                for w in range(nwt):
                    nc.tensor.matmul(oc[:, 0:8], Pv[:, :, w], stg[:, w, 0:8],
                                     start=(w == 0), stop=(w == nwt - 1))
                    nc.tensor.matmul(oc[:, 512:512 + DH], Pv[:, :, w],
                                     stg[:, w, 8:8 + DH],
                                     start=(w == 0), stop=(w == nwt - 1))
                den = smp.tile([128, 8], f32, tag="den1")
                nc.vector.tensor_scalar_max(den[:], oc[:, 0:8], 1e-30)
                rec = smp.tile([128, 8], f32, tag="rec1")
                nc.vector.reciprocal(rec[:], den[:])
                o1 = wkp.tile([128, DH], bf, tag="o1")
                nc.vector.tensor_tensor(
                    o1[:].rearrange("p (c h) -> p c h", h=8),
                    oc[:, 512:512 + DH].rearrange("p (c h) -> p c h", h=8),
                    rec[:].unsqueeze(1).broadcast_to([128, 64, 8]),
                    ALU.mult)
                if not b1_zero:
                    o1f = wkp.tile([128, DH], f32, tag="o1f")
                    nc.vector.tensor_add(o1f[:], o1[:], b1_sb[:])
                    o1 = o1f
                # ELU: h2 = relu(u) + exp(-relu(-u)) - 1
                pos = wkp.tile([128, DH], bf, tag="pos")
                nc.scalar.activation(pos[:], o1[:], AF.Relu)
                rneg = wkp.tile([128, DH], bf, tag="rneg")
                nc.scalar.activation(rneg[:], o1[:], AF.Relu, scale=-1.0)
                en = wkp.tile([128, DH], bf, tag="en")
                nc.scalar.activation(en[:], rneg[:], AF.Exp, scale=-1.0)
                h2 = wkp.tile([128, DH], bf, tag="h2")
                nc.vector.scalar_tensor_tensor(h2[:], pos[:], -1.0, en[:],
                                               ALU.add, ALU.add)
                # dense2
                hT2 = wkp.tile([128, 4, 128], bf, tag="hT2")
                pst = psZ.tile([128, 4, 128], bf, tag="zb1")
                for k in range(4):
                    nc.tensor.transpose(pst[:, k, :], h2[:, k * 128:(k + 1) * 128], ident[:])
                nc.scalar.copy(hT2[:], pst[:])
                ps2 = psD.tile([128, D + 2], f32, tag="ps2")
                for k in range(4):
                    nc.tensor.matmul(ps2[:], hT2[:, k, :], w2e[:, k, :],
                                     start=(k == 0), stop=(k == 3))
                nc.scalar.copy(adst2[:, t:t + 1], ps2[:, D + 1:D + 2])
                tb2 = iop.tile([128, ELEM2], bf, tag="tb2")
                nc.scalar.copy(tb2[:, 0:D + 1], ps2[:, 0:D + 1])
                nc.vector.memset(tb2[:, D + 1:D + 2], 1.0)
                qch = next(i for i in range(nch) if t < CHUNKT[i + 1])
                r0 = t * 128 - CHUNKB[qch]
                nc.sync.dma_start(tab2_locs[qch][r0:r0 + 128, 0:D + 2],
                                  tb2[:, 0:D + 2])
                if t == CHUNKT[qch + 1] - 1:
                    nc.gpsimd.collective_compute(
                        "AllGather", mybir.AluOpType.bypass,
                        replica_groups=[list(range(NCORE))],
                        ins=[tab2_locs[qch][:]],
                        outs=[tab2_fulls[qch].opt()],
                    )

            psC_cm.__exit__(None, None, None)
            psD_cm.__exit__(None, None, None)
            psZ_cm.__exit__(None, None, None)
            st_cm.__exit__(None, None, None)
            ga_cm.__exit__(None, None, None)
            sm_cm.__exit__(None, None, None)
            wk_cm.__exit__(None, None, None)
            phb.__exit__(None, None, None)

            # ================= PHASE C: conv2 edges =================
            phc = tc.tile_pool(name="phC", bufs=3)
            iop = phc.__enter__()
            sm_cm = tc.tile_pool(name="smC", bufs=3)
            smp = sm_cm.__enter__()
            ga_cm = tc.tile_pool(name="gaC", bufs=3)
            gap = ga_cm.__enter__()
            st_cm = tc.tile_pool(name="stC", bufs=2)
            stp = st_cm.__enter__()
            psZ_cm = tc.tile_pool(name="psZC", bufs=2, space="PSUM")
            psZ = psZ_cm.__enter__()
            psC_cm = tc.tile_pool(name="psCC", bufs=3, space="PSUM")
            psC = psC_cm.__enter__()
            for t in range(T):
                nws = [int(NW[t, r]) for r in range(nrun)]
                nwt = sum(nws)
                gt = gap.tile([128, NWT1, ELEM2], bf, tag="gt2")
                for (rbase, w0g, w0l, nw) in _calls(t, nws, Woff):
                    nc.gpsimd.dma_gather(
                        gt[:, w0l:w0l + nw, :], tab2_fulls[rbase][:],
                        idx_sb[:, w0g * 8:(w0g + nw) * 8],
                        num_idxs=nw * 128, num_idxs_reg=nw * 128,
                        elem_size=ELEM2, queue_num=nextq(),
                    )
                P = stp.tile([128, 128 * NWT1], bf, tag="P2")
                Pv = P[:, :128 * nwt].rearrange("p (s w) -> p s w", w=nwt)
                wb = 0
                for r in range(nrun):
                    if nws[r] == 0:
                        continue
                    sl_r = slots_sb[:, int(SWoff[t, r]):int(SWoff[t, r]) + nws[r]]
                    nc.vector.tensor_tensor(
                        Pv[:, :, wb:wb + nws[r]],
                        sl_r.unsqueeze(1).broadcast_to([128, 128, nws[r]]),
                        iotax3[:, :, 0:nws[r]], ALU.is_equal)
                    wb += nws[r]
                Pt = stp.tile([128, NWT1, 128], bf, tag="Pt2")
                m1 = stp.tile([128, NWT1, 128], bf, tag="m12")
                m2 = stp.tile([128, NWT1, 128], bf, tag="m22")
                wb = 0
                for r in range(nrun):
                    s0 = int(SWoff[t, r])
                    nw = nws[r]
                    iob = iotae[:].unsqueeze(1).broadcast_to([128, nw, 128])
                    nc.vector.tensor_tensor(
                        m1[:, wb:wb + nw, :], iob,
                        start_sb[:, s0:s0 + nw].unsqueeze(2)
                        .broadcast_to([128, nw, 128]), ALU.is_ge)
                    nc.vector.tensor_tensor(
                        m2[:, wb:wb + nw, :], iob,
                        end_sb[:, s0:s0 + nw].unsqueeze(2)
                        .broadcast_to([128, nw, 128]), ALU.is_ge)
                    nc.vector.tensor_tensor(
                        Pt[:, wb:wb + nw, :], m1[:, wb:wb + nw, :],
                        m2[:, wb:wb + nw, :], ALU.subtract)
                    wb += nws[r]
                zb = psZ.tile([128, NWT1], f32, tag="zb2")
                for w in range(nwt):
                    nc.tensor.matmul(zb[:, w:w + 1],
                                     Pt[:, w, :],
                                     adst2[:, t:t + 1],
                                     start=True, stop=True)
                z = smp.tile([128, NWT1], f32, tag="z2")
                nc.vector.scalar_tensor_tensor(
                    z[:, :nwt].rearrange("p (w d) -> p w d", d=1),
                    zb[:, :nwt].rearrange("p (w d) -> p w d", d=1), 1.0,
                    gt[:, 0:nwt, D:D + 1],
                    ALU.mult, ALU.add)
                e2 = smp.tile([128, NWT1], f32, tag="e22")
                nc.scalar.activation(e2[:, :nwt], z[:, :nwt], AF.Exp, scale=NEG)
                e1 = smp.tile([128, NWT1], f32, tag="e12")
                nc.scalar.activation(e1[:, :nwt], z[:, :nwt], AF.Exp)
                p2 = smp.tile([128, NWT1], bf, tag="p2")
                nc.vector.tensor_max(p2[:, :nwt], e1[:, :nwt], e2[:, :nwt])
                # fold attention weight into P (1 head): P~ = P * p2[e]
                nc.vector.tensor_tensor(
                    Pv[:, :, :], Pv[:, :, :],
                    p2[:, :nwt].unsqueeze(1).broadcast_to([128, 128, nwt]),
                    ALU.mult)
                # single matmul per window: cols 0:256 numerator, 257 denominator
                oc2 = psC.tile([128, D + 2], f32, tag="oc2")
                for w in range(nwt):
                    nc.tensor.matmul(oc2[:], Pv[:, :, w], gt[:, w, 0:D + 2],
                                     start=(w == 0), stop=(w == nwt - 1))
                den = smp.tile([128, 1], f32, tag="den2")
                nc.vector.tensor_scalar_max(den[:], oc2[:, D + 1:D + 2], 1e-30)
                rec = smp.tile([128, 1], f32, tag="rec2")
                nc.vector.reciprocal(rec[:], den[:])
                outt = iop.tile([128, D], f32, tag="outt")
                nc.vector.tensor_scalar(outt[:], oc2[:, 0:D], rec[:], None, ALU.mult)
                if not b2_zero:
                    nc.vector.tensor_add(outt[:], outt[:], b2_sb[:])
                nc.sync.dma_start(out_ext[t * 128:(t + 1) * 128, :], outt[:])
            psC_cm.__exit__(None, None, None)
            psZ_cm.__exit__(None, None, None)
            st_cm.__exit__(None, None, None)
            ga_cm.__exit__(None, None, None)
            sm_cm.__exit__(None, None, None)
            phc.__exit__(None, None, None)

    nc.compile()
    return nc


def _calls(t, nws, Woff):
    """Gather call plan for tile t: (run_base, global_w0, local_w0, nw)."""
    out = []
    lbase = 0
    for r, nwr in enumerate(nws):
        w0 = int(Woff[t, r])
        done = 0
        while done < nwr:
            nw = min(GCAP, nwr - done)
            out.append((r, w0 + done, lbase + done, nw))
            done += nw
        lbase += nwr
    return out


def _host_prep(inputs):
    edge_index = np.asarray(inputs["edge_index"])
    x = np.asarray(inputs["x"], np.float32)
    ln_w = np.asarray(inputs["ln_w"], np.float32)
    ln_b = np.asarray(inputs["ln_b"], np.float32)
    W1 = np.asarray(inputs["W1"], np.float32)
    a_s1 = np.asarray(inputs["att_src1"], np.float32)
    a_d1 = np.asarray(inputs["att_dst1"], np.float32)
    b1 = np.asarray(inputs["b1"], np.float32)
    W2 = np.asarray(inputs["W2"], np.float32)
    a_s2 = np.asarray(inputs["att_src2"], np.float32)
    a_d2 = np.asarray(inputs["att_dst2"], np.float32)
    b2 = np.asarray(inputs["b2"], np.float32)

    (NW, Woff, SWoff, Wtot, SWtot, idx_arrs, slot_arrs,
     start_arrs, end_arrs) = _prep_edges(edge_index)
    NWmax = int(NW.max())

    # W1_ext: c-major permuted cols + attention folds
    perm1 = np.empty(DH, np.int64)
    for h in range(H1):
        for c in range(C1):
            perm1[c * 8 + h] = h * C1 + c
    W1p = W1[:, perm1]
    wsrc1 = np.stack([W1[:, h * C1:(h + 1) * C1] @ a_s1[h] for h in range(H1)], 1)
    wdst1 = np.stack([W1[:, h * C1:(h + 1) * C1] @ a_d1[h] for h in range(H1)], 1)
    w1e = np.concatenate([W1p, wsrc1, wdst1], axis=1).astype(BF)  # [256, 528]

    # W2_ext: rows permuted to h2's c-major layout; cols natural; + att folds
    W2r = W2[perm1, :]
    wsrc2 = W2r @ a_s2[0]
    wdst2 = W2r @ a_d2[0]
    w2e = np.concatenate([W2r, wsrc2[:, None], wdst2[:, None]], axis=1).astype(BF)

    iotax = np.zeros((128, 128 * NWmax), np.float32)
    for s in range(128):
        iotax[:, s * NWmax:(s + 1) * NWmax] = s
    iotax = iotax.astype(BF)
    iotae = np.tile(np.arange(128, dtype=np.float32)[None, :], (128, 1)).astype(BF)
    identm = np.eye(128).astype(BF)

    ln_trivial = bool(np.all(ln_w == 1.0) and np.all(ln_b == 0.0))
    b1_zero = bool(np.all(b1 == 0.0))
    b2_zero = bool(np.all(b2 == 0.0))

    in_maps = []
    for c in range(NCORE):
        xp = np.zeros((NPAD, D), np.float32)
        xp[:NLOC] = x[c * NLOC:(c + 1) * NLOC]
        m = {
            "x": xp, "idx": idx_arrs[c], "slots": slot_arrs[c],
            "startv": start_arrs[c], "endv": end_arrs[c],
            "w1e": w1e, "w2e": w2e, "iotax": iotax, "iotae": iotae,
            "ident": identm,
        }
        if not ln_trivial:
            m["lnw"] = np.tile(ln_w[None, :], (128, 1)).astype(np.float32)
            m["lnb"] = np.tile(ln_b[None, :], (128, 1)).astype(np.float32)
        if not b1_zero:
            m["b1r"] = np.tile(b1[perm1][None, :], (128, 1)).astype(np.float32)
        if not b2_zero:
            m["b2r"] = np.tile(b2[None, :], (128, 1)).astype(np.float32)
        in_maps.append(m)
    meta = (NW, Woff, SWoff, Wtot, SWtot, ln_trivial, b1_zero, b2_zero)
    return meta, in_maps


def kernel(**inputs):
    _install_ntff_hook()
    from concourse.bass_utils import run_bass_kernel_spmd

    meta, in_maps = _host_prep(inputs)
    NW, Woff, SWoff, Wtot, SWtot, ln_trivial, b1_zero, b2_zero = meta
    key = (Wtot, SWtot, ln_trivial, b1_zero, b2_zero, NW.tobytes())
    if key not in _cache:
        _cache[key] = _build(NW, Woff, SWoff, Wtot, SWtot,
                             ln_trivial, b1_zero, b2_zero)
    nc = _cache[key]

    trace = bool(int(__import__("os").environ.get("KERNEL_TRACE", "0")))
    res = run_bass_kernel_spmd(nc, in_maps, core_ids=list(range(NCORE)),
                               trace=trace)
    kernel.last_exec_time_ns = res.exec_time_ns
    out = np.concatenate([res.results[c]["out"][:NLOC] for c in range(NCORE)], 0)
    return out.astype(np.float32)


kernel.last_exec_time_ns = None


# revision 21
# speedup vs baseline: 1.1550x; 1.0065x over previous
"""Distributed 2-layer GAT (nn_AlignHead) on 8 TRN2 NeuronCores.

Strategy: shard nodes (dst) contiguously across 8 cores. Per core:
  Phase A: LayerNorm + h@W1_ext dense matmul -> per-node table rows
           [g1 (c-major, 512) | a_src1 (8) | pad] bf16; chunked AllGather
           (uneven chunks: big chunk overlaps compute, small tail chunk).
  Phase B: per dst-tile (128 dsts): dma_gather edge src rows, segment
           softmax via indicator matmuls. P[e,s] built on DVE (is_equal);
           P^T built DIRECTLY on DVE from host staircase bounds (edges are
           slot-sorted per window => P^T rows are column ranges:
           (iota>=start)*(iota<end), 2 DVE ops). a_dst broadcast via
           P^T-matmul, p = max(exp(z), exp(0.2 z)), weighted aggregation +
           denominator fused in ONE 520-col matmul per window, normalize,
           ELU -> h2, dense h2@W2_ext -> table2; chunked AllGather.
  Phase C: conv2 edge phase (1 head): attention scalar folded INTO P
           (per-partition scale), single 258-col matmul per window with a
           ones-column denominator; normalize -> output rows.

Self-contained: hardcodes the problem shapes; compiles on first call.
"""
import sys
import types

import numpy as np
import ml_dtypes

# ---------------------------------------------------------------- constants
NCORE = 8
N = 50000
E = 500000
D = 256
H1, C1 = 8, 64
DH = 512            # H1*C1
NEG = 0.2
EPS = 1e-5
NLOC = 6250         # nodes per core
NPAD = 6272         # 49*128
T = 49              # dst tiles per core
ROWS = NPAD * NCORE  # 50176 global (padded) table rows
LO = 32768          # int16 gather row limit
ELEM1 = 640         # bf16 elems per conv1 table row (1280 B)
ELEM2 = 384         # bf16 elems per conv2 table row (768 B)
NQ = 4              # swdge queues
# AG chunks double as the int16 gather-range split: chunk q's global table
# [8*(CB[q+1]-CB[q]) rows] fits int16 indexing entirely. Uneven on purpose:
# the big chunk's AllGather overlaps compute; only the small tail blocks.
CHUNKB = [0, 3968, 6272]   # AG chunk boundaries (local rows)  8*3968=31744<32768
CHUNKT = [0, 31, 49]       # tile boundaries per chunk
BF = ml_dtypes.bfloat16
GCAP = 4            # max windows per dma_gather call (512 descs fit the
                    # SWDGE ring; bigger calls hit superlinear DGE stalls)

_cache = {}


def _install_ntff_hook():
    if "antenv.axon_hooks" in sys.modules:
        return
    try:
        import antenv
        mod = types.ModuleType("antenv.axon_hooks")
        _h = [None]
        mod.set_axon_ntff_profile_hook = lambda h: _h.__setitem__(0, h)
        mod.get_axon_ntff_profile_hook = lambda: _h[0]
        sys.modules["antenv.axon_hooks"] = mod
        antenv.axon_hooks = mod
        from trn_agent_boot.trn_boot import _ntff_profile_via_ctypes
        mod.set_axon_ntff_profile_hook(
            _ntff_profile_via_ctypes("/opt/axon/libaxon_pjrt.so"))
    except Exception:
        pass


def _prep_edges(edge_index):
    """Partition + window-pad edges. Returns (NW [T,nrun], Woff, SWoff, Wtot,
    SWtot, per-core idx arrays [128, Wtot*8] int16, slot arrays [128, SWtot]
    bf16, staircase start/end arrays [128, SWtot] bf16)."""
    src = np.asarray(edge_index[0]).astype(np.int64)
    dst = np.asarray(edge_index[1]).astype(np.int64)
    loops = np.arange(N, dtype=np.int64)
    src = np.concatenate([src, loops])
    dst = np.concatenate([dst, loops])
    # the appended self-loops form their own run: exactly one edge per slot,
    # slot-sorted => identity window, filled by a DENSE local DMA (no gather
    # descriptors at all).
    is_self = np.zeros(len(src), bool)
    is_self[E:] = True

    core = dst // NLOC
    ldst = dst % NLOC
    tilei = ldst // 128
    slot = ldst % 128
    # per-chunk global table layout: chunk q holds local rows
    # [CB[q], CB[q+1]) of every core, as its own (Shared) tensor.
    # run = chunk index; idx values are chunk-local (always < 32768).
    s_core = src // NLOC
    s_loc = src % NLOC
    q = np.searchsorted(np.asarray(CHUNKB[1:-1]), s_loc, side="right")
    cb = np.asarray(CHUNKB)[q]
    csz = (np.asarray(CHUNKB[1:]) - np.asarray(CHUNKB[:-1]))[q]
    srow = s_core * csz + (s_loc - cb)   # chunk-local row
    nchk = len(CHUNKB) - 1
    run = np.where(is_self, nchk, q)     # self-loops: run index nchk

    nrun = nchk + 1
    cnt = np.zeros((NCORE, T, nrun), np.int64)
    np.add.at(cnt, (core, tilei, run), 1)
    NW = np.maximum(1, np.ceil(cnt.max(axis=0) / 128).astype(np.int64))  # [T,nrun]
    nexact = cnt.max(axis=0)             # exact idx count per (t, run)

    Woff = np.zeros((T, nrun), np.int64)
    w = 0
    for t in range(T):
        for r in range(nrun):
            Woff[t, r] = w
            w += NW[t, r]
    Wtot = int(w)

    # slot array column offsets: per (t) aligned to even
    SWoff = np.zeros((T, nrun), np.int64)
    sw = 0
    for t in range(T):
        for r in range(nrun):
            SWoff[t, r] = sw
            sw += NW[t, r] + (NW[t, r] & 1)
    SWtot = int(sw)

    order = np.lexsort((ldst, run, tilei, core))
    src_s = srow[order]
    core_s = core[order]
    tile_s = tilei[order]
    run_s = run[order]
    slot_s = slot[order]

    idx_arrs, slot_arrs, start_arrs, end_arrs = [], [], [], []
    # per (core,t,r) segment boundaries in the sorted arrays
    seg_key = ((core_s * T + tile_s) * nrun + run_s)
    bounds = np.searchsorted(seg_key, np.arange(NCORE * T * nrun + 1))
    srange = np.arange(128)
    for c in range(NCORE):
        idx16 = np.zeros((16, Wtot * 8), np.int16)
        slots = np.full((128, SWtot), 128.0, np.float32)
        startv = np.zeros((128, SWtot), np.float32)
        endv = np.zeros((128, SWtot), np.float32)
        for t in range(T):
            for r in range(nrun):
                k = (c * T + t) * nrun + r
                a, b = bounds[k], bounds[k + 1]
                n = b - a
                nw = int(NW[t, r])
                assert n <= nw * 128
                rows = src_s[a:b]
                sl = slot_s[a:b]
                j = np.arange(n)
                w0 = int(Woff[t, r])
                idx16[j % 16, w0 * 8 + j // 16] = rows.astype(np.int16)
                s0 = int(SWoff[t, r])
                slots[j % 128, s0 + j // 128] = sl
                # staircase bounds for direct P^T build: within window w the
                # edges are slot-sorted, so P^T[s,:] is the col range
                # [start_s, end_s).
                for w in range(nw):
                    wsl = sl[w * 128:(w + 1) * 128]
                    startv[:, s0 + w] = np.searchsorted(wsl, srange, "left")
                    endv[:, s0 + w] = np.searchsorted(wsl, srange, "right")
        idx_arrs.append(np.tile(idx16, (8, 1)))
        slot_arrs.append(slots.astype(BF))
        start_arrs.append(startv.astype(BF))
        end_arrs.append(endv.astype(BF))
    return (NW, Woff, SWoff, Wtot, SWtot, nexact,
            idx_arrs, slot_arrs, start_arrs, end_arrs)


def _build(NW, Woff, SWoff, Wtot, SWtot, nexact, ln_trivial, b1_zero, b2_zero):
    import concourse.bacc as bacc
    import concourse.mybir as mybir
    import concourse.tile as tile

    f32 = mybir.dt.float32
    bf = mybir.dt.bfloat16
    i16 = mybir.dt.int16
    AF = mybir.ActivationFunctionType
    ALU = mybir.AluOpType
    NWmax = int(NW.max())
    NWT1 = int(NW.sum(axis=1).max())

    nc = bacc.Bacc("TRN2", target_bir_lowering=False, debug=False,
                   num_devices=NCORE, num_swdge_queues=NQ)

    x_in = nc.declare_dram_parameter("x", [NPAD, D], f32, isOutput=False)
    idx_in = nc.declare_dram_parameter("idx", [128, Wtot * 8], i16, isOutput=False)
    sl_in = nc.declare_dram_parameter("slots", [128, SWtot], bf, isOutput=False)
    st_in = nc.declare_dram_parameter("startv", [128, SWtot], bf, isOutput=False)
    en_in = nc.declare_dram_parameter("endv", [128, SWtot], bf, isOutput=False)
    w1_in = nc.declare_dram_parameter("w1e", [D, DH + 16], bf, isOutput=False)
    w2_in = nc.declare_dram_parameter("w2e", [DH, D + 2], bf, isOutput=False)
    io_in = nc.declare_dram_parameter("iotax", [128, 128 * NWmax], bf, isOutput=False)
    ie_in = nc.declare_dram_parameter("iotae", [128, 128], bf, isOutput=False)
    id_in = nc.declare_dram_parameter("ident", [128, 128], bf, isOutput=False)
    lnw_in = lnb_in = b1_in = b2_in = None
    if not ln_trivial:
        lnw_in = nc.declare_dram_parameter("lnw", [128, D], f32, isOutput=False)
        lnb_in = nc.declare_dram_parameter("lnb", [128, D], f32, isOutput=False)
    if not b1_zero:
        b1_in = nc.declare_dram_parameter("b1r", [128, DH], f32, isOutput=False)
    if not b2_zero:
        b2_in = nc.declare_dram_parameter("b2r", [128, D], f32, isOutput=False)
    out_ext = nc.declare_dram_parameter("out", [NPAD, D], f32, isOutput=True)

    nch = len(CHUNKB) - 1
    nrun = nch + 1   # gather runs + the self-loop run (dense DMA, no gather)
    tab1_locs = [nc.dram_tensor(f"tab1_loc{q}", [CHUNKB[q + 1] - CHUNKB[q], ELEM1], bf)
                 for q in range(nch)]
    tab2_locs = [nc.dram_tensor(f"tab2_loc{q}", [CHUNKB[q + 1] - CHUNKB[q], ELEM2], bf)
                 for q in range(nch)]

    qrot = [0]

    def nextq():
        q = qrot[0]
        qrot[0] = (q + 1) % NQ
        return q

    with tile.TileContext(nc) as tc:
        with (
            tc.tile_pool(name="const", bufs=1) as cpool,
            tc.tile_pool(name="dram", bufs=1, space="DRAM") as dpool,
        ):
            tab1_fulls = [dpool.tile([NCORE * (CHUNKB[q + 1] - CHUNKB[q]), ELEM1],
                                     bf, addr_space="Shared", name=f"t1f{q}")
                          for q in range(nch)]
            tab2_fulls = [dpool.tile([NCORE * (CHUNKB[q + 1] - CHUNKB[q]), ELEM2],
                                     bf, addr_space="Shared", name=f"t2f{q}")
                          for q in range(nch)]

            # ---- constants to SBUF
            w1e = cpool.tile([128, 2, DH + 16], bf)
            nc.sync.dma_start(w1e[:], w1_in[:].rearrange("(k p) f -> p k f", p=128))
            w2e = cpool.tile([128, 4, D + 2], bf)
            nc.sync.dma_start(w2e[:], w2_in[:].rearrange("(k p) f -> p k f", p=128))
            iotax = cpool.tile([128, 128 * NWmax], bf)
            nc.sync.dma_start(iotax[:], io_in[:])
            iotae = cpool.tile([128, 128], bf)
            nc.sync.dma_start(iotae[:], ie_in[:])
            ident = cpool.tile([128, 128], bf)
            nc.sync.dma_start(ident[:], id_in[:])
            slots_sb = cpool.tile([128, SWtot], bf)
            nc.sync.dma_start(slots_sb[:], sl_in[:])
            start_sb = cpool.tile([128, SWtot], bf)
            nc.sync.dma_start(start_sb[:], st_in[:])
            end_sb = cpool.tile([128, SWtot], bf)
            nc.sync.dma_start(end_sb[:], en_in[:])
            idx_sb = cpool.tile([128, Wtot * 8], i16)
            nc.sync.dma_start(idx_sb[:], idx_in[:])
            adst1 = cpool.tile([128, T * 8], bf)
            adst2 = cpool.tile([128, T], bf)
            if not ln_trivial:
                lnw_sb = cpool.tile([128, D], f32)
                nc.sync.dma_start(lnw_sb[:], lnw_in[:])
                lnb_sb = cpool.tile([128, D], f32)
                nc.sync.dma_start(lnb_sb[:], lnb_in[:])
            if not b1_zero:
                b1_sb = cpool.tile([128, DH], f32)
                nc.sync.dma_start(b1_sb[:], b1_in[:])
            if not b2_zero:
                b2_sb = cpool.tile([128, D], f32)
                nc.sync.dma_start(b2_sb[:], b2_in[:])

            iotax3 = iotax[:].rearrange("p (s w) -> p s w", w=NWmax)

            # ================= PHASE A: LN + dense1 + table1 =================
            pha = tc.tile_pool(name="phA", bufs=4)
            iop = pha.__enter__()
            wk_cm = tc.tile_pool(name="wkA", bufs=3)
            wkp = wk_cm.__enter__()
            sm_cm = tc.tile_pool(name="smA", bufs=4)
            smp = sm_cm.__enter__()
            psA_cm = tc.tile_pool(name="psA", bufs=3, space="PSUM")
            psA = psA_cm.__enter__()
            psT_cm = tc.tile_pool(name="psTA", bufs=2, space="PSUM")
            psT = psT_cm.__enter__()
            for t in range(T):
                xt = iop.tile([128, D], f32, tag="xt")
                nc.sync.dma_start(xt[:], x_in[t * 128:(t + 1) * 128, :])
                mean = smp.tile([128, 1], f32, tag="mean")
                nc.vector.reduce_sum(mean[:], xt[:], axis=mybir.AxisListType.X)
                nc.vector.tensor_scalar_mul(mean[:], mean[:], 1.0 / D)
                xc = wkp.tile([128, D], f32, tag="xc")
                nc.vector.tensor_scalar(xc[:], xt[:], mean[:], None, ALU.subtract)
                sq = smp.tile([128, 1], f32, tag="sq")
                sqj = wkp.tile([128, D], f32, tag="sqj")
                nc.scalar.activation(sqj[:], xc[:], AF.Square, accum_out=sq[:])
                nc.vector.tensor_scalar(sq[:], sq[:], 1.0 / D, EPS, ALU.mult, ALU.add)
                sd = smp.tile([128, 1], f32, tag="sd")
                nc.scalar.activation(sd[:], sq[:], AF.Sqrt)
                rstd = smp.tile([128, 1], f32, tag="rstd")
                nc.vector.reciprocal(rstd[:], sd[:])
                hbf = wkp.tile([128, D], bf, tag="hbf")
                if ln_trivial:
                    nc.scalar.activation(hbf[:], xc[:], AF.Copy, scale=rstd[:])
                else:
                    hf = wkp.tile([128, D], f32, tag="hf")
                    nc.scalar.activation(hf[:], xc[:], AF.Copy, scale=rstd[:])
                    nc.vector.tensor_mul(hf[:], hf[:], lnw_sb[:])
                    nc.vector.tensor_add(hbf[:], hf[:], lnb_sb[:])
                # transpose h -> [feat, node]
                hT = wkp.tile([128, 2, 128], bf, tag="hT")
                for k in range(2):
                    pst = psT.tile([128, 128], bf, tag="pstA")
                    nc.tensor.transpose(pst[:], hbf[:, k * 128:(k + 1) * 128], ident[:])
                    nc.scalar.copy(hT[:, k, :], pst[:])
                ps1 = psA.tile([128, DH], f32, tag="ps1")
                ps1b = psA.tile([128, 16], f32, tag="ps1b")
                for k in range(2):
                    nc.tensor.matmul(ps1[:], hT[:, k, :], w1e[:, k, 0:DH],
                                     start=(k == 0), stop=(k == 1))
                    nc.tensor.matmul(ps1b[:], hT[:, k, :],
                                     w1e[:, k, DH:DH + 16],
                                     start=(k == 0), stop=(k == 1))
                nc.scalar.copy(adst1[:, t * 8:(t + 1) * 8], ps1b[:, 8:16])
                tb = iop.tile([128, ELEM1], bf, tag="tb1")
                # ps1 is already c-major (W1e columns pre-permuted on host)
                nc.scalar.copy(tb[:, 0:DH], ps1[:])
                nc.scalar.copy(tb[:, DH:DH + 8], ps1b[:, 0:8])
                qch = next(i for i in range(nch) if t < CHUNKT[i + 1])
                r0 = t * 128 - CHUNKB[qch]
                nc.sync.dma_start(tab1_locs[qch][r0:r0 + 128, 0:DH + 8],
                                  tb[:, 0:DH + 8])
                if t == CHUNKT[qch + 1] - 1:
                    nc.gpsimd.collective_compute(
                        "AllGather", mybir.AluOpType.bypass,
                        replica_groups=[list(range(NCORE))],
                        ins=[tab1_locs[qch][:]],
                        outs=[tab1_fulls[qch].opt()],
                    )

            psT_cm.__exit__(None, None, None)
            psA_cm.__exit__(None, None, None)
            sm_cm.__exit__(None, None, None)
            wk_cm.__exit__(None, None, None)
            pha.__exit__(None, None, None)

            # ================= PHASE B: conv1 edges + dense2 =================
            phb = tc.tile_pool(name="phB", bufs=3)
            iop = phb.__enter__()
            wk_cm = tc.tile_pool(name="wkB", bufs=2)
            wkp = wk_cm.__enter__()
            sm_cm = tc.tile_pool(name="smB", bufs=3)
            smp = sm_cm.__enter__()
            ga_cm = tc.tile_pool(name="gaB", bufs=3)
            gap = ga_cm.__enter__()
            st_cm = tc.tile_pool(name="stB", bufs=2)
            stp = st_cm.__enter__()
            psZ_cm = tc.tile_pool(name="psZ", bufs=2, space="PSUM")
            psZ = psZ_cm.__enter__()
            psD_cm = tc.tile_pool(name="psD", bufs=2, space="PSUM")
            psD = psD_cm.__enter__()
            psC_cm = tc.tile_pool(name="psC", bufs=2, space="PSUM")
            psC = psC_cm.__enter__()
            for t in range(T):
                nws = [int(NW[t, r]) for r in range(nrun)]
                nwt = sum(nws)
                ws = nwt - 1   # self-loop window (identity, dense DMA)
                gt = gap.tile([128, NWT1, ELEM1], bf, tag="gt1")
                for (rbase, w0g, w0l, nw, nidx) in _calls(t, nws[:nch], Woff, nexact):
                    nc.gpsimd.dma_gather(
                        gt[:, w0l:w0l + nw, :], tab1_fulls[rbase][:],
                        idx_sb[:, w0g * 8:(w0g + nw) * 8],
                        num_idxs=nidx, num_idxs_reg=nidx,
                        elem_size=ELEM1, queue_num=nextq(),
                    )
                qch = next(i for i in range(nch) if t < CHUNKT[i + 1])
                r0s = t * 128 - CHUNKB[qch]
                nc.sync.dma_start(gt[:, ws, 0:DH + 8],
                                  tab1_locs[qch][r0s:r0s + 128, 0:DH + 8])
                P = stp.tile([128, 128 * NWT1], bf, tag="P1")
                Pv = P[:, :128 * nwt].rearrange("p (s w) -> p s w", w=nwt)
                wb = 0
                for r in range(nrun):
                    if nws[r] == 0:
                        continue
                    sl_r = slots_sb[:, int(SWoff[t, r]):int(SWoff[t, r]) + nws[r]]
                    nc.vector.tensor_tensor(
                        Pv[:, :, wb:wb + nws[r]],
                        sl_r.unsqueeze(1).broadcast_to([128, 128, nws[r]]),
                        iotax3[:, :, 0:nws[r]], ALU.is_equal)
                    wb += nws[r]
                # P^T built directly on DVE from staircase bounds:
                # Pt[s, w, j] = (j >= start[s,w]) - (j >= end[s,w])
                Pt = stp.tile([128, NWT1, 128], bf, tag="Pt1")
                m1 = stp.tile([128, NWT1, 128], bf, tag="m11")
                m2 = stp.tile([128, NWT1, 128], bf, tag="m21")
                wb = 0
                for r in range(nrun):
                    s0 = int(SWoff[t, r])
                    nw = nws[r]
                    iob = iotae[:].unsqueeze(1).broadcast_to([128, nw, 128])
                    nc.vector.tensor_tensor(
                        m1[:, wb:wb + nw, :], iob,
                        start_sb[:, s0:s0 + nw].unsqueeze(2)
                        .broadcast_to([128, nw, 128]), ALU.is_ge)
                    nc.vector.tensor_tensor(
                        m2[:, wb:wb + nw, :], iob,
                        end_sb[:, s0:s0 + nw].unsqueeze(2)
                        .broadcast_to([128, nw, 128]), ALU.is_ge)
                    nc.vector.tensor_tensor(
                        Pt[:, wb:wb + nw, :], m1[:, wb:wb + nw, :],
                        m2[:, wb:wb + nw, :], ALU.subtract)
                    wb += nws[r]
                zb = psZ.tile([128, NWT1 * 8], f32, tag="zb1")
                for w in range(nwt):
                    nc.tensor.matmul(zb[:, w * 8:(w + 1) * 8],
                                     Pt[:, w, :],
                                     adst1[:, t * 8:(t + 1) * 8],
                                     start=True, stop=True)
                z = smp.tile([128, NWT1 * 8], f32, tag="z1")
                nc.vector.scalar_tensor_tensor(
                    z[:, :nwt * 8].rearrange("p (w d) -> p w d", d=8),
                    zb[:, :nwt * 8].rearrange("p (w d) -> p w d", d=8), 1.0,
                    gt[:, 0:nwt, DH:DH + 8],
                    ALU.mult, ALU.add)
                e2 = smp.tile([128, NWT1 * 8], f32, tag="e21")
                nc.scalar.activation(e2[:, :nwt * 8], z[:, :nwt * 8], AF.Exp, scale=NEG)
                e1 = smp.tile([128, NWT1 * 8], f32, tag="e11")
                nc.scalar.activation(e1[:, :nwt * 8], z[:, :nwt * 8], AF.Exp)
                stg = stp.tile([128, NWT1, 8 + DH], bf, tag="stg1")
                nc.vector.tensor_tensor(
                    stg[:, 0:nwt, 0:8],
                    e1[:, :nwt * 8].rearrange("p (w d) -> p w d", d=8),
                    e2[:, :nwt * 8].rearrange("p (w d) -> p w d", d=8),
                    ALU.max)
                # W'' = g (c-major) * p-bcast
                nc.vector.tensor_mul(
                    stg[:, 0:nwt, 8:8 + DH].rearrange("p w (c h) -> p w c h", h=8),
                    gt[:, 0:nwt, 0:DH].rearrange("p w (c h) -> p w c h", h=8),
                    stg[:, 0:nwt, 0:8].unsqueeze(2).broadcast_to([128, nwt, 64, 8]))
                # denominator (bank 0, cols 0:8) + numerator (bank 1): one
                # matmul may not cross a PSUM bank boundary
                oc = psC.tile([128, 1024], f32, tag="oc1")
                for w in range(nwt):
                    nc.tensor.matmul(oc[:, 0:8], Pv[:, :, w], stg[:, w, 0:8],
                                     start=(w == 0), stop=(w == nwt - 1))
                    nc.tensor.matmul(oc[:, 512:512 + DH], Pv[:, :, w],
                                     stg[:, w, 8:8 + DH],
                                     start=(w == 0), stop=(w == nwt - 1))
                den = smp.tile([128, 8], f32, tag="den1")
                nc.vector.tensor_scalar_max(den[:], oc[:, 0:8], 1e-30)
                rec = smp.tile([128, 8], f32, tag="rec1")
                nc.vector.reciprocal(rec[:], den[:])
                o1 = wkp.tile([128, DH], bf, tag="o1")
                nc.vector.tensor_tensor(
                    o1[:].rearrange("p (c h) -> p c h", h=8),
                    oc[:, 512:512 + DH].rearrange("p (c h) -> p c h", h=8),
                    rec[:].unsqueeze(1).broadcast_to([128, 64, 8]),
                    ALU.mult)
                if not b1_zero:
                    o1f = wkp.tile([128, DH], f32, tag="o1f")
                    nc.vector.tensor_add(o1f[:], o1[:], b1_sb[:])
                    o1 = o1f
                # ELU: h2 = relu(u) + exp(-relu(-u)) - 1
                pos = wkp.tile([128, DH], bf, tag="pos")
                nc.scalar.activation(pos[:], o1[:], AF.Relu)
                rneg = wkp.tile([128, DH], bf, tag="rneg")
                nc.scalar.activation(rneg[:], o1[:], AF.Relu, scale=-1.0)
                en = wkp.tile([128, DH], bf, tag="en")
                nc.scalar.activation(en[:], rneg[:], AF.Exp, scale=-1.0)
                h2 = wkp.tile([128, DH], bf, tag="h2")
                nc.vector.scalar_tensor_tensor(h2[:], pos[:], -1.0, en[:],
                                               ALU.add, ALU.add)
                # dense2
                hT2 = wkp.tile([128, 4, 128], bf, tag="hT2")
                pst = psZ.tile([128, 4, 128], bf, tag="zb1")
                for k in range(4):
                    nc.tensor.transpose(pst[:, k, :], h2[:, k * 128:(k + 1) * 128], ident[:])
                nc.scalar.copy(hT2[:], pst[:])
                ps2 = psD.tile([128, D + 2], f32, tag="ps2")
                for k in range(4):
                    nc.tensor.matmul(ps2[:], hT2[:, k, :], w2e[:, k, :],
                                     start=(k == 0), stop=(k == 3))
                nc.scalar.copy(adst2[:, t:t + 1], ps2[:, D + 1:D + 2])
                tb2 = iop.tile([128, ELEM2], bf, tag="tb2")
                nc.scalar.copy(tb2[:, 0:D + 1], ps2[:, 0:D + 1])
                nc.vector.memset(tb2[:, D + 1:D + 2], 1.0)
                qch = next(i for i in range(nch) if t < CHUNKT[i + 1])
                r0 = t * 128 - CHUNKB[qch]
                nc.sync.dma_start(tab2_locs[qch][r0:r0 + 128, 0:D + 2],
                                  tb2[:, 0:D + 2])
                if t == CHUNKT[qch + 1] - 1:
                    nc.gpsimd.collective_compute(
                        "AllGather", mybir.AluOpType.bypass,
                        replica_groups=[list(range(NCORE))],
                        ins=[tab2_locs[qch][:]],
                        outs=[tab2_fulls[qch].opt()],
                    )

            psC_cm.__exit__(None, None, None)
            psD_cm.__exit__(None, None, None)
            psZ_cm.__exit__(None, None, None)
            st_cm.__exit__(None, None, None)
            ga_cm.__exit__(None, None, None)
            sm_cm.__exit__(None, None, None)
            wk_cm.__exit__(None, None, None)
            phb.__exit__(None, None, None)

            # ================= PHASE C: conv2 edges =================
            phc = tc.tile_pool(name="phC", bufs=3)
            iop = phc.__enter__()
            sm_cm = tc.tile_pool(name="smC", bufs=3)
            smp = sm_cm.__enter__()
            ga_cm = tc.tile_pool(name="gaC", bufs=3)
            gap = ga_cm.__enter__()
            st_cm = tc.tile_pool(name="stC", bufs=2)
            stp = st_cm.__enter__()
            psQ_cm = tc.tile_pool(name="psQC", bufs=2, space="PSUM")
            psQ = psQ_cm.__enter__()
            psZ_cm = tc.tile_pool(name="psZC", bufs=2, space="PSUM")
            psZ = psZ_cm.__enter__()
            psC_cm = tc.tile_pool(name="psCC", bufs=2, space="PSUM")
            psC = psC_cm.__enter__()
            for t in range(T):
                nws = [int(NW[t, r]) for r in range(nrun)]
                nwt = sum(nws)
                ws = nwt - 1
                gt = gap.tile([128, NWT1, ELEM2], bf, tag="gt2")
                for (rbase, w0g, w0l, nw, nidx) in _calls(t, nws[:nch], Woff, nexact):
                    nc.gpsimd.dma_gather(
                        gt[:, w0l:w0l + nw, :], tab2_fulls[rbase][:],
                        idx_sb[:, w0g * 8:(w0g + nw) * 8],
                        num_idxs=nidx, num_idxs_reg=nidx,
                        elem_size=ELEM2, queue_num=nextq(),
                    )
                qch = next(i for i in range(nch) if t < CHUNKT[i + 1])
                r0s = t * 128 - CHUNKB[qch]
                nc.sync.dma_start(gt[:, ws, 0:D + 2],
                                  tab2_locs[qch][r0s:r0s + 128, 0:D + 2])
                P = stp.tile([128, 128 * NWT1], bf, tag="P2")
                Pv = P[:, :128 * nwt].rearrange("p (s w) -> p s w", w=nwt)
                wb = 0
                for r in range(nrun):
                    if nws[r] == 0:
                        continue
                    sl_r = slots_sb[:, int(SWoff[t, r]):int(SWoff[t, r]) + nws[r]]
                    nc.vector.tensor_tensor(
                        Pv[:, :, wb:wb + nws[r]],
                        sl_r.unsqueeze(1).broadcast_to([128, 128, nws[r]]),
                        iotax3[:, :, 0:nws[r]], ALU.is_equal)
                    wb += nws[r]
                Qp = psQ.tile([128, NWT1 * 128], bf, tag="Qp2")
                for w in range(nwt):
                    nc.tensor.transpose(Qp[:, w * 128:(w + 1) * 128],
                                        Pv[:, :, w], ident[:])
                Q = stp.tile([128, NWT1 * 128], bf, tag="Q2")
                nc.scalar.copy(Q[:, :nwt * 128], Qp[:, :nwt * 128])
                zb = psZ.tile([128, NWT1], f32, tag="zb2")
                for w in range(nwt):
                    nc.tensor.matmul(zb[:, w:w + 1],
                                     Q[:, w * 128:(w + 1) * 128],
                                     adst2[:, t:t + 1],
                                     start=True, stop=True)
                z = smp.tile([128, NWT1], f32, tag="z2")
                nc.vector.scalar_tensor_tensor(
                    z[:, :nwt].rearrange("p (w d) -> p w d", d=1),
                    zb[:, :nwt].rearrange("p (w d) -> p w d", d=1), 1.0,
                    gt[:, 0:nwt, D:D + 1],
                    ALU.mult, ALU.add)
                e2 = smp.tile([128, NWT1], f32, tag="e22")
                nc.scalar.activation(e2[:, :nwt], z[:, :nwt], AF.Exp, scale=NEG)
                e1 = smp.tile([128, NWT1], f32, tag="e12")
                nc.scalar.activation(e1[:, :nwt], z[:, :nwt], AF.Exp)
                p2 = smp.tile([128, NWT1], bf, tag="p2")
                nc.vector.tensor_max(p2[:, :nwt], e1[:, :nwt], e2[:, :nwt])
                # fold attention weight into P (1 head): P~ = P * p2[e]
                nc.vector.tensor_tensor(
                    Pv[:, :, :], Pv[:, :, :],
                    p2[:, :nwt].unsqueeze(1).broadcast_to([128, 128, nwt]),
                    ALU.mult)
                # single matmul per window: cols 0:256 numerator, 257 denominator
                oc2 = psC.tile([128, D + 2], f32, tag="oc2")
                for w in range(nwt):
                    nc.tensor.matmul(oc2[:], Pv[:, :, w], gt[:, w, 0:D + 2],
                                     start=(w == 0), stop=(w == nwt - 1))
                den = smp.tile([128, 1], f32, tag="den2")
                nc.vector.tensor_scalar_max(den[:], oc2[:, D + 1:D + 2], 1e-30)
                rec = smp.tile([128, 1], f32, tag="rec2")
                nc.vector.reciprocal(rec[:], den[:])
                outt = iop.tile([128, D], f32, tag="outt")
                nc.vector.tensor_scalar(outt[:], oc2[:, 0:D], rec[:], None, ALU.mult)
                if not b2_zero:
                    nc.vector.tensor_add(outt[:], outt[:], b2_sb[:])
                nc.sync.dma_start(out_ext[t * 128:(t + 1) * 128, :], outt[:])
            psC_cm.__exit__(None, None, None)
            psZ_cm.__exit__(None, None, None)
            psQ_cm.__exit__(None, None, None)
            st_cm.__exit__(None, None, None)
            ga_cm.__exit__(None, None, None)
            sm_cm.__exit__(None, None, None)
            phc.__exit__(None, None, None)

    nc.compile()
    return nc


def _calls(t, nws, Woff, nexact):
    """Gather call plan for tile t: (run_base, global_w0, local_w0, nw, nidx).

    Only gather runs (not the self run). nidx is the exact descriptor count
    for the call (max over cores); the first 3 tiles gather fully padded so
    ring-buffer reuse guarantees stale SBUF windows hold valid floats.
    """
    out = []
    lbase = 0
    for r, nwr in enumerate(nws):
        w0 = int(Woff[t, r])
        ntot = int(nexact[t, r]) if t >= 3 else nwr * 128
        done = 0
        while done < nwr:
            nw = min(GCAP, nwr - done)
            nidx = min(nw * 128, max(1, ntot - done * 128))
            out.append((r, w0 + done, lbase + done, nw, nidx))
            done += nw
        lbase += nwr
    return out


def _host_prep(inputs):
    edge_index = np.asarray(inputs["edge_index"])
    x = np.asarray(inputs["x"], np.float32)
    ln_w = np.asarray(inputs["ln_w"], np.float32)
    ln_b = np.asarray(inputs["ln_b"], np.float32)
    W1 = np.asarray(inputs["W1"], np.float32)
    a_s1 = np.asarray(inputs["att_src1"], np.float32)
    a_d1 = np.asarray(inputs["att_dst1"], np.float32)
    b1 = np.asarray(inputs["b1"], np.float32)
    W2 = np.asarray(inputs["W2"], np.float32)
    a_s2 = np.asarray(inputs["att_src2"], np.float32)
    a_d2 = np.asarray(inputs["att_dst2"], np.float32)
    b2 = np.asarray(inputs["b2"], np.float32)

    (NW, Woff, SWoff, Wtot, SWtot, nexact, idx_arrs, slot_arrs,
     start_arrs, end_arrs) = _prep_edges(edge_index)
    NWmax = int(NW.max())

    # W1_ext: c-major permuted cols + attention folds
    perm1 = np.empty(DH, np.int64)
    for h in range(H1):
        for c in range(C1):
            perm1[c * 8 + h] = h * C1 + c
    W1p = W1[:, perm1]
    wsrc1 = np.stack([W1[:, h * C1:(h + 1) * C1] @ a_s1[h] for h in range(H1)], 1)
    wdst1 = np.stack([W1[:, h * C1:(h + 1) * C1] @ a_d1[h] for h in range(H1)], 1)
    w1e = np.concatenate([W1p, wsrc1, wdst1], axis=1).astype(BF)  # [256, 528]

    # W2_ext: rows permuted to h2's c-major layout; cols natural; + att folds
    W2r = W2[perm1, :]
    wsrc2 = W2r @ a_s2[0]
    wdst2 = W2r @ a_d2[0]
    w2e = np.concatenate([W2r, wsrc2[:, None], wdst2[:, None]], axis=1).astype(BF)

    iotax = np.zeros((128, 128 * NWmax), np.float32)
    for s in range(128):
        iotax[:, s * NWmax:(s + 1) * NWmax] = s
    iotax = iotax.astype(BF)
    iotae = np.tile(np.arange(128, dtype=np.float32)[None, :], (128, 1)).astype(BF)
    identm = np.eye(128).astype(BF)

    ln_trivial = bool(np.all(ln_w == 1.0) and np.all(ln_b == 0.0))
    b1_zero = bool(np.all(b1 == 0.0))
    b2_zero = bool(np.all(b2 == 0.0))

    in_maps = []
    for c in range(NCORE):
        xp = np.zeros((NPAD, D), np.float32)
        xp[:NLOC] = x[c * NLOC:(c + 1) * NLOC]
        m = {
            "x": xp, "idx": idx_arrs[c], "slots": slot_arrs[c],
            "startv": start_arrs[c], "endv": end_arrs[c],
            "w1e": w1e, "w2e": w2e, "iotax": iotax, "iotae": iotae,
            "ident": identm,
        }
        if not ln_trivial:
            m["lnw"] = np.tile(ln_w[None, :], (128, 1)).astype(np.float32)
            m["lnb"] = np.tile(ln_b[None, :], (128, 1)).astype(np.float32)
        if not b1_zero:
            m["b1r"] = np.tile(b1[perm1][None, :], (128, 1)).astype(np.float32)
        if not b2_zero:
            m["b2r"] = np.tile(b2[None, :], (128, 1)).astype(np.float32)
        in_maps.append(m)
    meta = (NW, Woff, SWoff, Wtot, SWtot, nexact, ln_trivial, b1_zero, b2_zero)
    return meta, in_maps


def kernel(**inputs):
    _install_ntff_hook()
    from concourse.bass_utils import run_bass_kernel_spmd

    meta, in_maps = _host_prep(inputs)
    NW, Woff, SWoff, Wtot, SWtot, nexact, ln_trivial, b1_zero, b2_zero = meta
    key = (Wtot, SWtot, ln_trivial, b1_zero, b2_zero, NW.tobytes(),
           nexact.tobytes())
    if key not in _cache:
        _cache[key] = _build(NW, Woff, SWoff, Wtot, SWtot, nexact,
                             ln_trivial, b1_zero, b2_zero)
    nc = _cache[key]

    trace = bool(int(__import__("os").environ.get("KERNEL_TRACE", "0")))
    res = run_bass_kernel_spmd(nc, in_maps, core_ids=list(range(NCORE)),
                               trace=trace)
    kernel.last_exec_time_ns = res.exec_time_ns
    out = np.concatenate([res.results[c]["out"][:NLOC] for c in range(NCORE)], 0)
    return out.astype(np.float32)


kernel.last_exec_time_ns = None
